# revision 1
# baseline (speedup 1.0000x reference)
"""Trainium2 Bass kernel for nn_Attention_86638080295542.

Multi-head attention (12 heads, d=64) with the reference's v=k quirk:
    q = x @ Wq.T + bq ; k = x @ Wk.T + bk ; v = k
    out = softmax(q k^T / sqrt(d)) @ v ;  y = out @ Wo.T + bo

Sharding: batch (B=8) data-parallel across the 8 NeuronCores — core c
computes batch element c end-to-end, no collectives.

Per-core dataflow (all "T" tensors keep the contraction dim on SBUF
partitions so every matmul is a natural lhsT.T @ rhs):
  xT[e,s], WqT/WkT/WoT[e_in,e_out] are pre-transposed on the host.
  qT = Wq @ xT   (+bq per-partition)        [768,1024]
  kT = Wk @ xT   (+bk per-partition)        [768,1024]
  vaug[j, jb, h, 0:64] = k natural (PE transpose of kT), col 64 = 1.0
  per head h: pT[j,i] = exp(scale * kT_h^T qT_h)  (no max-subtraction:
     logits are O(1) for this problem, softmax is shift-invariant)
  outT_h[d,i] (+ rowsum in row 64) = vaug^T @ pT, accumulated over j
  normalize: outT_h *= 1/rowsum (broadcast via ones-matmul on PE)
  y = outT^T @ WoT + bo
"""

from contextlib import ExitStack

import numpy as np

import concourse.bass as bass
import concourse.tile as tile
from concourse import bacc, mybir
from concourse import bass_utils

S = 1024          # sequence length
E = 768           # embed dim
H = 12            # heads
DH = 64           # head dim
P = 128           # partitions
KT = E // P       # 6 k-tiles over embed dim
ST = S // P       # 8 tiles over sequence
NCH = S // 512    # 2 free-dim chunks of 512 over sequence
SCALE = DH ** -0.5
NCORES = 8

F32 = mybir.dt.float32
F32R = mybir.dt.float32r
BF16 = mybir.dt.bfloat16


def _emit(nc, tc, ctx, iters=1, variant='full'):
    xT_d = nc.dram_tensor("xT", [E, S], F32R, kind="ExternalInput")
    WqT_d = nc.dram_tensor("WqT", [E, E], F32R, kind="ExternalInput")
    WkT_d = nc.dram_tensor("WkT", [E, E], F32R, kind="ExternalInput")
    WoT_d = nc.dram_tensor("WoT", [E, E], F32R, kind="ExternalInput")
    bq_d = nc.dram_tensor("bq", [E], F32, kind="ExternalInput")
    bk_d = nc.dram_tensor("bk", [E], F32, kind="ExternalInput")
    bo_d = nc.dram_tensor("bo", [E], F32, kind="ExternalInput")
    y_d = nc.dram_tensor("y", [S, E], F32, kind="ExternalOutput")

    Exp = mybir.ActivationFunctionType.Exp

    const = ctx.enter_context(tc.tile_pool(name="const", bufs=1))
    xt_pool = ctx.enter_context(tc.tile_pool(name="xt", bufs=1))
    outt_pool = ctx.enter_context(tc.tile_pool(name="outt", bufs=1))
    w_pool = ctx.enter_context(tc.tile_pool(name="w", bufs=2))
    wo_pool = ctx.enter_context(tc.tile_pool(name="wo", bufs=1))
    vaug_pool = ctx.enter_context(tc.tile_pool(name="vaug", bufs=1))
    qt_pool = ctx.enter_context(tc.tile_pool(name="qt", bufs=3))
    kt_pool = ctx.enter_context(tc.tile_pool(name="kt", bufs=3))
    pt_pool = ctx.enter_context(tc.tile_pool(name="pt", bufs=8))
    ysb_pool = ctx.enter_context(tc.tile_pool(name="ysb", bufs=2))
    pvsb_pool = ctx.enter_context(tc.tile_pool(name="pvsb", bufs=4))
    rc_pool = ctx.enter_context(tc.tile_pool(name="rc", bufs=2))
    rb_pool = ctx.enter_context(tc.tile_pool(name="rb", bufs=2))
    ps_s = ctx.enter_context(tc.tile_pool(name="ps_s", bufs=2, space="PSUM"))
    ps_pv = ctx.enter_context(tc.tile_pool(name="ps_pv", bufs=2, space="PSUM"))
    dram_pool = ctx.enter_context(tc.tile_pool(name="dram", bufs=4, space="DRAM"))

    if iters > 1:
        ctx.enter_context(tc.For_i(0, iters, 1))

    # ---- constants ----
    # gpsimd/memset can't emit float32r, so build fp32 then copy-round on DVE
    # (0.0/1.0 are exactly representable, so the copy is exact).
    ident_f32 = const.tile([P, P], F32, tag="ident_f32")
    from concourse.masks import make_identity
    make_identity(nc, ident_f32[:])
    identity = const.tile([P, P], F32R, tag="ident")
    nc.vector.tensor_copy(identity[:], ident_f32[:])
    ones64_f32 = const.tile([1, DH], F32, tag="ones64_f32")
    nc.vector.memset(ones64_f32[:], 1.0)
    ones64 = const.tile([1, DH], F32R, tag="ones64")
    nc.vector.tensor_copy(ones64[:], ones64_f32[:])
    bq_sb = const.tile([P, KT], F32, tag="bq")
    nc.sync.dma_start(bq_sb[:], bq_d.ap().rearrange("(t p) -> p t", p=P))
    bk_sb = const.tile([P, KT], F32, tag="bk")
    nc.sync.dma_start(bk_sb[:], bk_d.ap().rearrange("(t p) -> p t", p=P))
    # bo broadcast to all 128 partitions via a 0-step partition AP (DRAM APs
    # are not partitioned, so a 0-step leading dim is legal here)
    bo_bc = const.tile([P, E], F32, tag="bo")
    bo_ap = bo_d.ap()
    bo_bcast_src = bass.AP(bo_ap.tensor, bo_ap.offset, [[0, P], [1, E]])
    nc.sync.dma_start(bo_bc[:], bo_bcast_src)

    # ---- input loads (per k-tile so compute can start early) ----
    xT_sb = xt_pool.tile([P, KT, S], F32R, tag="xt")
    WqT_sb = w_pool.tile([P, KT, E], F32R, tag="w")
    WkT_sb = w_pool.tile([P, KT, E], F32R, tag="w")
    WoT_sb = wo_pool.tile([P, KT, E], F32R, tag="wo")
    xT_r = xT_d.ap().rearrange("(t p) s -> p t s", p=P)
    WqT_r = WqT_d.ap().rearrange("(t p) e -> p t e", p=P)
    WkT_r = WkT_d.ap().rearrange("(t p) e -> p t e", p=P)
    WoT_r = WoT_d.ap().rearrange("(t p) e -> p t e", p=P)
    for t in range(KT):
        nc.sync.dma_start(xT_sb[:, t, :], xT_r[:, t, :])
        nc.sync.dma_start(WqT_sb[:, t, :], WqT_r[:, t, :])
        nc.sync.dma_start(WkT_sb[:, t, :], WkT_r[:, t, :])
        nc.sync.dma_start(WoT_sb[:, t, :], WoT_r[:, t, :])

    vaug = vaug_pool.tile([P, ST, H, DH + 1], BF16, tag="vaug")
    for jb in range(ST):
        nc.vector.memset(vaug[:, jb, :, DH:DH + 1], 1.0)
    outT_sb = outt_pool.tile([P, KT, S], F32R, tag="outt")

    # ---- per head-pair: projections (tq=hp), vaug transposes (t=hp),
    # then the pair's attention. Interleaving lets ACT exp work start while
    # the PE is still projecting later tiles, overlapping the two engines.
    for hp in range(KT):
        # projections for e-tile hp: qT/kT rows [128*hp, 128*hp+128), written
        # into per-pair rotating tiles (only this pair ever reads them)
        qp = qt_pool.tile([P, S], F32R, tag="qt", name=f"qp_{hp}")
        kp = kt_pool.tile([P, S], F32R, tag="kt", name=f"kp_{hp}")
        if variant == "onlyheads":
            nc.vector.memset(qp[:].bitcast(F32), 0.01)
            nc.vector.memset(kp[:].bitcast(F32), 0.01)
        else:
            for W_sb, b_sb, out_sb in ((WqT_sb, bq_sb, qp), (WkT_sb, bk_sb, kp)):
                for c in range(NCH):
                    ps = ps_s.tile([P, 512], F32, tag="ps_s")
                    for t in range(KT):
                        nc.tensor.matmul(
                            ps[:],
                            W_sb[:, t, 128 * hp:128 * hp + 128],
                            xT_sb[:, t, 512 * c:512 * c + 512],
                            start=(t == 0), stop=(t == KT - 1),
                        )
                    nc.vector.tensor_scalar_add(
                        out_sb[:, 512 * c:512 * c + 512], ps[:], b_sb[:, hp:hp + 1]
                    )
        # vaug slices for heads 2hp, 2hp+1 via PE transposes of kT tile hp
        if variant == "onlyheads":
            if hp == 0:
                nc.vector.memset(vaug[:, :, :, 0:DH], 0.01)
        else:
            for g in range(2):
                ps = ps_s.tile([P, 512], F32R, tag="ps_s")
                for j4 in range(4):
                    jb = 4 * g + j4
                    nc.tensor.transpose(
                        ps[:, 128 * j4:128 * j4 + 128],
                        kp[:, 128 * jb:128 * jb + 128],
                        identity[:],
                    )
                nc.vector.tensor_copy(
                    vaug[:, 4 * g:4 * g + 4, 2 * hp:2 * hp + 2, 0:DH],
                    ps[:].rearrange("p (a b c) -> p a b c", a=4, b=2, c=DH),
                )
        # attention for the two heads of this pair, one head at a time.
        # Score psums are triple-buffered [128, S] tiles so the PE can run
        # a couple of j-blocks ahead of the ACT exp evictions.
        if variant == "noheads":
            for h in (2 * hp, 2 * hp + 1):
                po = DH * (h % 2)
                nc.vector.memset(outT_sb[po:po + DH, hp, :].bitcast(F32), 0.01)
            continue
        for h in (2 * hp, 2 * hp + 1):
            po = DH * (h % 2)
            pv = ps_pv.tile([DH + 1, S], F32, tag="ps_pv", name=f"pv_{h}")

            def pv_mms(jb, pt):
                for c in range(NCH):
                    nc.tensor.matmul(
                        pv[:, 512 * c:512 * c + 512],
                        vaug[:, jb, h, :],
                        pt[:, 512 * c:512 * c + 512],
                        start=(jb == 0), stop=(jb == ST - 1),
                    )

            # software-pipelined by one j-block: the PE issues scores(jb)
            # before PV(jb-1), so exp(jb-1) on ACT overlaps scores(jb) on PE
            # instead of stalling the PE.
            prev = None
            for jb in range(ST):
                sps = ps_s.tile([P, S], F32, tag="ps_s", name=f"sps_{h}_{jb}")
                for c in range(NCH):
                    nc.tensor.matmul(
                        sps[:, 512 * c:512 * c + 512],
                        kp[po:po + DH, 128 * jb:128 * jb + 128],
                        qp[po:po + DH, 512 * c:512 * c + 512],
                        start=True, stop=True,
                    )
                pt = pt_pool.tile([P, S], BF16, tag="pt")
                nc.scalar.activation(pt[:], sps[:], Exp, scale=SCALE)
                if prev is not None:
                    pv_mms(jb - 1, prev)
                prev = pt
            pv_mms(ST - 1, prev)
            # evict pv to SBUF right away (frees the PSUM bank), then
            # normalize: reciprocal of the rowsum row, broadcast across 64
            # partitions via a DRAM round-trip (DRAM APs allow a 0-step
            # partition dim), multiply into outT. Keeps the PE entirely out
            # of the normalization chain.
            pvsb = pvsb_pool.tile([DH + 1, S], F32, tag="pvsb", name=f"pvsb_{h}")
            nc.vector.tensor_copy(pvsb[:], pv[:])
            rc = rc_pool.tile([1, S], F32, tag="rc", name=f"rc_{h}")
            nc.vector.reciprocal(rc[:], pvsb[DH:DH + 1, :])
            rd = dram_pool.tile([1, S], F32, tag="rd", name=f"rd_{h}")
            nc.sync.dma_start(rd[:], rc[:])
            rb = rb_pool.tile([DH, S], F32, tag="rb", name=f"rb_{h}")
            rd_ap = rd[:]
            nc.sync.dma_start(
                rb[:], bass.AP(rd_ap.tensor, rd_ap.offset, [[0, DH], [1, S]]))
            nc.vector.tensor_mul(
                outT_sb[po:po + DH, hp, :], pvsb[0:DH, :], rb[:],
            )

    # ---- output projection: y = outT^T @ WoT + bo ----
    if variant == "onlyheads":
        nc.sync.dma_start(
            y_d.ap().rearrange("(a b) e -> a (b e)", a=P),
            outT_sb[:].rearrange("p t s -> p (t s)").bitcast(F32),
        )
        return
    y_r = y_d.ap().rearrange("(st p) e -> st p e", p=P)
    for st in range(ST):
        ysb = ysb_pool.tile([P, E], F32, tag="ysb")
        for n0 in (0, 384):
            yps = ps_s.tile([P, 512], F32, tag="ps_s")
            for t in range(KT):
                nc.tensor.matmul(
                    yps[:, 0:384],
                    outT_sb[:, t, 128 * st:128 * st + 128],
                    WoT_sb[:, t, n0:n0 + 384],
                    start=(t == 0), stop=(t == KT - 1),
                )
            nc.vector.tensor_add(ysb[:, n0:n0 + 384], yps[:, 0:384], bo_bc[:, n0:n0 + 384])
        nc.sync.dma_start(y_r[st], ysb[:])


_NC_CACHE = {}


def build(iters=1, variant="full"):
    key = (iters, variant)
    nc = _NC_CACHE.get(key)
    if nc is None:
        nc = bacc.Bacc("TRN2", target_bir_lowering=False, debug=False)
        with tile.TileContext(nc) as tc, ExitStack() as ctx:
            _emit(nc, tc, ctx, iters=iters, variant=variant)
        nc.compile()
        _NC_CACHE[key] = nc
    return nc


def _round_tf32(a):
    """Round fp32 to tf32 (10 explicit mantissa bits), RNE, fp32 container."""
    a = np.ascontiguousarray(np.asarray(a, dtype=np.float32))
    u = a.view(np.uint32)
    lsb = (u >> np.uint32(13)) & np.uint32(1)
    r = (u + np.uint32(0x0FFF) + lsb) & np.uint32(0xFFFFE000)
    return r.view(np.float32)


def make_in_maps(x, Wq, bq, Wk, bk, Wo, bo):
    WqT = _round_tf32(np.asarray(Wq, dtype=np.float32).T)
    WkT = _round_tf32(np.asarray(Wk, dtype=np.float32).T)
    WoT = _round_tf32(np.asarray(Wo, dtype=np.float32).T)
    bq = np.ascontiguousarray(np.asarray(bq, dtype=np.float32))
    bk = np.ascontiguousarray(np.asarray(bk, dtype=np.float32))
    bo = np.ascontiguousarray(np.asarray(bo, dtype=np.float32))
    x = np.asarray(x, dtype=np.float32)
    return [
        {
            "xT": _round_tf32(x[c].T),
            "WqT": WqT, "WkT": WkT, "WoT": WoT,
            "bq": bq, "bk": bk, "bo": bo,
        }
        for c in range(NCORES)
    ]


def kernel(x, Wq, bq, Wk, bk, Wo, bo):
    nc = build()
    in_maps = make_in_maps(x, Wq, bq, Wk, bk, Wo, bo)
    res = bass_utils.run_bass_kernel_spmd(nc, in_maps, core_ids=list(range(NCORES)))
    return np.stack([res.results[c]["y"] for c in range(NCORES)]).astype(np.float32)



# revision 4
# speedup vs baseline: 1.1331x; 1.1331x over previous
"""Trainium2 Bass kernel for nn_Attention_86638080295542.

Multi-head attention (12 heads, d=64) with the reference's v=k quirk:
    q = x @ Wq.T + bq ; k = x @ Wk.T + bk ; v = k
    out = softmax(q k^T / sqrt(d)) @ v ;  y = out @ Wo.T + bo

Sharding: batch (B=8) data-parallel across the 8 NeuronCores — core c
computes batch element c end-to-end, no collectives.

Per-core dataflow (all "T" tensors keep the contraction dim on SBUF
partitions so every matmul is a natural lhsT.T @ rhs):
  xT[e,s], WqT/WkT/WoT[e_in,e_out] are pre-transposed on the host.
  qT = Wq @ xT   (+bq per-partition)        [768,1024]
  kT = Wk @ xT   (+bk per-partition)        [768,1024]
  vaug[j, jb, h, 0:64] = k natural (PE transpose of kT), col 64 = 1.0
  per head h: pT[j,i] = exp(scale * kT_h^T qT_h)  (no max-subtraction:
     logits are O(1) for this problem, softmax is shift-invariant)
  outT_h[d,i] (+ rowsum in row 64) = vaug^T @ pT, accumulated over j
  normalize: outT_h *= 1/rowsum (rowsum broadcast across the 64
     partitions via gpsimd partition_broadcast)
  y = outT^T @ WoT + bo

Schedule: the attention inner loop is ACT(exp)-paced (exp of a
[128,512] chunk takes ~610ns vs ~430ns of PE work), so the PE work for
the next pair's projections and this pair's vaug transposes is split
into ~2-matmul "pieces" and one piece is emitted per j-block inside the
attention loop, keeping the PE busy while ACT drains.

PSUM budget (8 banks): scores ring 3x[128,512] (3) + proj/trans slot
1x[128,512] (1) + two PV accumulators [65,1024] (4).
"""

from contextlib import ExitStack

import numpy as np

import concourse.bass as bass
import concourse.tile as tile
from concourse import bacc, mybir
from concourse import bass_utils

S = 1024          # sequence length
E = 768           # embed dim
H = 12            # heads
DH = 64           # head dim
P = 128           # partitions
KT = E // P       # 6 k-tiles over embed dim
ST = S // P       # 8 tiles over sequence
NCH = S // 512    # 2 free-dim chunks of 512 over sequence
SCALE = DH ** -0.5
NCORES = 8

F32 = mybir.dt.float32
F32R = mybir.dt.float32r
BF16 = mybir.dt.bfloat16

# rowsum broadcast: 'gpsimd' = nc.gpsimd.partition_broadcast,
# 'dma' = DRAM round-trip with a 0-step partition AP
BCAST = 'gpsimd'


def _emit(nc, tc, ctx, iters=1, variant='full'):
    xT_d = nc.dram_tensor("xT", [E, S], F32R, kind="ExternalInput")
    WqT_d = nc.dram_tensor("WqT", [E, E], F32R, kind="ExternalInput")
    WkT_d = nc.dram_tensor("WkT", [E, E], F32R, kind="ExternalInput")
    WoT_d = nc.dram_tensor("WoT", [E, E], F32R, kind="ExternalInput")
    bq_d = nc.dram_tensor("bq", [E], F32, kind="ExternalInput")
    bk_d = nc.dram_tensor("bk", [E], F32, kind="ExternalInput")
    bo_d = nc.dram_tensor("bo", [E], F32, kind="ExternalInput")
    y_d = nc.dram_tensor("y", [S, E], F32, kind="ExternalOutput")

    Exp = mybir.ActivationFunctionType.Exp

    const = ctx.enter_context(tc.tile_pool(name="const", bufs=1))
    xt_pool = ctx.enter_context(tc.tile_pool(name="xt", bufs=1))
    outt_pool = ctx.enter_context(tc.tile_pool(name="outt", bufs=1))
    wq_pool = ctx.enter_context(tc.tile_pool(name="wq", bufs=1))
    wk_pool = ctx.enter_context(tc.tile_pool(name="wk", bufs=1))
    wo_pool = ctx.enter_context(tc.tile_pool(name="wo", bufs=1))
    vaug_pool = ctx.enter_context(tc.tile_pool(name="vaug", bufs=1))
    qt_pool = ctx.enter_context(tc.tile_pool(name="qt", bufs=3))
    kt_pool = ctx.enter_context(tc.tile_pool(name="kt", bufs=3))
    pt_pool = ctx.enter_context(tc.tile_pool(name="pt", bufs=6))
    ysb_pool = ctx.enter_context(tc.tile_pool(name="ysb", bufs=4))
    rc_pool = ctx.enter_context(tc.tile_pool(name="rc", bufs=2))
    rb_pool = ctx.enter_context(tc.tile_pool(name="rb", bufs=2))
    ps_sc = ctx.enter_context(tc.tile_pool(name="ps_sc", bufs=3, space="PSUM"))
    ps_w = ctx.enter_context(tc.tile_pool(name="ps_w", bufs=1, space="PSUM"))
    ps_pv = ctx.enter_context(tc.tile_pool(name="ps_pv", bufs=2, space="PSUM"))
    if BCAST == 'dma':
        dram_pool = ctx.enter_context(
            tc.tile_pool(name="dram", bufs=4, space="DRAM"))

    # ---- loop-invariant constants (outside the timing loop) ----
    ident_f32 = const.tile([P, P], F32, tag="ident_f32")
    from concourse.masks import make_identity
    make_identity(nc, ident_f32[:])
    identity = const.tile([P, P], F32R, tag="ident")
    nc.vector.tensor_copy(identity[:], ident_f32[:])
    bq_sb = const.tile([P, KT], F32, tag="bq")
    nc.sync.dma_start(bq_sb[:], bq_d.ap().rearrange("(t p) -> p t", p=P))
    bk_sb = const.tile([P, KT], F32, tag="bk")
    nc.sync.dma_start(bk_sb[:], bk_d.ap().rearrange("(t p) -> p t", p=P))
    # bo broadcast to all 128 partitions via a 0-step partition AP (DRAM APs
    # are not partitioned, so a 0-step leading dim is legal here)
    bo_bc = const.tile([P, E], F32, tag="bo")
    bo_ap = bo_d.ap()
    bo_bcast_src = bass.AP(bo_ap.tensor, bo_ap.offset, [[0, P], [1, E]])
    nc.sync.dma_start(bo_bc[:], bo_bcast_src)

    if iters > 1:
        ctx.enter_context(tc.For_i(0, iters, 1))

    # ---- input loads: xT first (needed in full by proj 0), Wq/Wk in
    # hp-column slices so proj hp only waits for slice hp, WoT last ----
    xT_sb = xt_pool.tile([P, KT, S], F32R, tag="xt")
    WqT_sb = wq_pool.tile([P, KT, E], F32R, tag="wq")
    WkT_sb = wk_pool.tile([P, KT, E], F32R, tag="wk")
    WoT_sb = wo_pool.tile([P, KT, E], F32R, tag="wo")
    xT_r = xT_d.ap().rearrange("(t p) s -> p t s", p=P)
    WqT_r = WqT_d.ap().rearrange("(t p) e -> p t e", p=P)
    WkT_r = WkT_d.ap().rearrange("(t p) e -> p t e", p=P)
    WoT_r = WoT_d.ap().rearrange("(t p) e -> p t e", p=P)
    nc.sync.dma_start(xT_sb[:, 0, :], xT_r[:, 0, :])
    nc.sync.dma_start(WqT_sb[:, :, 0:P], WqT_r[:, :, 0:P])
    nc.sync.dma_start(WkT_sb[:, :, 0:P], WkT_r[:, :, 0:P])
    for t in range(1, KT):
        nc.sync.dma_start(xT_sb[:, t, :], xT_r[:, t, :])
    for hp in range(1, KT):
        c0, c1 = P * hp, P * hp + P
        nc.sync.dma_start(WqT_sb[:, :, c0:c1], WqT_r[:, :, c0:c1])
        nc.sync.dma_start(WkT_sb[:, :, c0:c1], WkT_r[:, :, c0:c1])
    for t in range(KT):
        nc.sync.dma_start(WoT_sb[:, t, :], WoT_r[:, t, :])

    vaug = vaug_pool.tile([P, ST, H, DH + 1], BF16, tag="vaug")
    for jb in range(ST):
        nc.vector.memset(vaug[:, jb, :, DH:DH + 1], 1.0)
    outT_sb = outt_pool.tile([P, KT, S], F32R, tag="outt")

    qps = [None] * KT
    kps = [None] * KT

    def proj_pieces(hp, which, c):
        """One projection chunk as 3 pieces of 2 accumulating mms each; the
        last piece evicts to SBUF with the bias add (DVE)."""
        W_sb, b_sb = (WqT_sb, bq_sb) if which == 'q' else (WkT_sb, bk_sb)
        out_sb = qps[hp] if which == 'q' else kps[hp]
        st = {}

        def piece(tp, first, last):
            def go():
                if first:
                    st['ps'] = ps_w.tile([P, 512], F32, tag="ps_w",
                                         name=f"pj_{which}{hp}_{c}")
                ps = st['ps']
                for t in tp:
                    nc.tensor.matmul(
                        ps[:],
                        W_sb[:, t, 128 * hp:128 * hp + 128],
                        xT_sb[:, t, 512 * c:512 * c + 512],
                        start=(t == 0), stop=(t == KT - 1),
                    )
                if last:
                    nc.vector.tensor_scalar_add(
                        out_sb[:, 512 * c:512 * c + 512], ps[:],
                        b_sb[:, hp:hp + 1])
            return go

        return [piece((0, 1), True, False), piece((2, 3), False, False),
                piece((4, 5), False, True)]

    def trans_pieces(hp, g):
        """4 PE transposes of kp(hp) block g + DVE copy into vaug, as 2
        pieces of 2 transposes each."""
        st = {}

        def piece(j4s, first, last):
            def go():
                if first:
                    st['ps'] = ps_w.tile([P, 512], F32R, tag="ps_w",
                                         name=f"tr_{hp}_{g}")
                ps = st['ps']
                kp = kps[hp]
                for j4 in j4s:
                    jb = 4 * g + j4
                    nc.tensor.transpose(
                        ps[:, 128 * j4:128 * j4 + 128],
                        kp[:, 128 * jb:128 * jb + 128],
                        identity[:],
                    )
                if last:
                    nc.vector.tensor_copy(
                        vaug[:, 4 * g:4 * g + 4, 2 * hp:2 * hp + 2, 0:DH],
                        ps[:].rearrange("p (a b c) -> p a b c", a=4, b=2, c=DH),
                    )
            return go

        return [piece((0, 1), True, False), piece((2, 3), False, True)]

    def alloc_qk(hp):
        qps[hp] = qt_pool.tile([P, S], F32R, tag="qt", name=f"qp_{hp}")
        kps[hp] = kt_pool.tile([P, S], F32R, tag="kt", name=f"kp_{hp}")

    def head_attn(hp, h, pre, fill):
        """Attention for head h of pair hp. `pre` pieces run before the
        j-loop; `fill` pieces are consumed one per j-block inside it."""
        po = DH * (h % 2)
        qp, kp = qps[hp], kps[hp]
        pv = ps_pv.tile([DH + 1, S], F32, tag="ps_pv", name=f"pv_{h}")
        for f in pre:
            f()

        def pv_mms(jb, pt):
            for c in range(NCH):
                nc.tensor.matmul(
                    pv[:, 512 * c:512 * c + 512],
                    vaug[:, jb, h, :],
                    pt[:, 512 * c:512 * c + 512],
                    start=(jb == 0), stop=(jb == ST - 1),
                )

        fi = 0
        prev = None
        for jb in range(ST):
            pt = pt_pool.tile([P, S], BF16, tag="pt")
            for c in range(NCH):
                sps = ps_sc.tile([P, 512], F32, tag="ps_sc",
                                 name=f"sps_{h}_{jb}_{c}")
                nc.tensor.matmul(
                    sps[:],
                    kp[po:po + DH, 128 * jb:128 * jb + 128],
                    qp[po:po + DH, 512 * c:512 * c + 512],
                    start=True, stop=True,
                )
                nc.scalar.activation(
                    pt[:, 512 * c:512 * c + 512], sps[:], Exp, scale=SCALE)
            if fi < len(fill):
                fill[fi]()
                fi += 1
            if prev is not None:
                pv_mms(jb - 1, prev)
            prev = pt
        pv_mms(ST - 1, prev)
        for f in fill[fi:]:
            f()

        # normalization, chunked so the chain latency is ~half a tile:
        # rc = 1/rowsum (DVE, straight off PSUM), broadcast across the 64
        # head-dim partitions on gpsimd, multiply into outT (DVE).
        rc = rc_pool.tile([1, S], F32, tag="rc", name=f"rc_{h}")
        rb = rb_pool.tile([DH, S], F32, tag="rb", name=f"rb_{h}")
        if BCAST == 'dma':
            rd = dram_pool.tile([1, S], F32, tag="rd", name=f"rd_{h}")
        for c in range(NCH):
            cs = slice(512 * c, 512 * c + 512)
            nc.vector.reciprocal(rc[:, cs], pv[DH:DH + 1, cs])
            if BCAST == 'gpsimd':
                nc.gpsimd.partition_broadcast(rb[:, cs], rc[:, cs])
            else:
                nc.sync.dma_start(rd[:, cs], rc[:, cs])
                rd_ap = rd[:, cs]
                nc.sync.dma_start(
                    rb[:, cs],
                    bass.AP(rd_ap.tensor, rd_ap.offset, [[0, DH], [1, 512]]))
        for c in range(NCH):
            cs = slice(512 * c, 512 * c + 512)
            nc.vector.tensor_mul(
                outT_sb[po:po + DH, hp, cs], pv[0:DH, cs], rb[:, cs])

    # ---- pair 0 projections (no attention to hide them under) ----
    alloc_qk(0)
    for pc in (proj_pieces(0, 'q', 0) + proj_pieces(0, 'q', 1)
               + proj_pieces(0, 'k', 0) + proj_pieces(0, 'k', 1)
               + trans_pieces(0, 0) + trans_pieces(0, 1)):
        pc()

    # ---- pairs: attention with next pair's projections as in-loop filler ----
    for hp in range(KT):
        tg0 = trans_pieces(hp, 0) if hp > 0 else []
        tg1 = trans_pieces(hp, 1) if hp > 0 else []
        if hp + 1 < KT:
            alloc_qk(hp + 1)
            q0 = proj_pieces(hp + 1, 'q', 0)
            q1 = proj_pieces(hp + 1, 'q', 1)
            k0 = proj_pieces(hp + 1, 'k', 0)
            k1 = proj_pieces(hp + 1, 'k', 1)
        else:
            q0 = q1 = k0 = k1 = []
        head_attn(hp, 2 * hp, tg0, tg1 + q0 + q1)
        head_attn(hp, 2 * hp + 1, [], k0 + k1)

    # ---- output projection: y = outT^T @ WoT + bo ----
    y_r = y_d.ap().rearrange("(st p) e -> st p e", p=P)
    for st in range(ST):
        ysb = ysb_pool.tile([P, E], F32, tag="ysb")
        for n0 in (0, 384):
            yps = ps_sc.tile([P, 512], F32, tag="ps_sc", name=f"yp_{st}_{n0}")
            for t in range(KT):
                nc.tensor.matmul(
                    yps[:, 0:384],
                    outT_sb[:, t, 128 * st:128 * st + 128],
                    WoT_sb[:, t, n0:n0 + 384],
                    start=(t == 0), stop=(t == KT - 1),
                )
            nc.vector.tensor_add(ysb[:, n0:n0 + 384], yps[:, 0:384],
                                 bo_bc[:, n0:n0 + 384])
        # stores ride the ACT hwdge queue so next iteration's input loads
        # on the sync queue are not serialized behind them
        nc.scalar.dma_start(y_r[st], ysb[:])


_NC_CACHE = {}


def build(iters=1, variant="full"):
    key = (iters, variant)
    nc = _NC_CACHE.get(key)
    if nc is None:
        nc = bacc.Bacc("TRN2", target_bir_lowering=False, debug=False)
        with tile.TileContext(nc) as tc, ExitStack() as ctx:
            _emit(nc, tc, ctx, iters=iters, variant=variant)
        nc.compile()
        _NC_CACHE[key] = nc
    return nc


def _round_tf32(a):
    """Round fp32 to tf32 (10 explicit mantissa bits), RNE, fp32 container."""
    a = np.ascontiguousarray(np.asarray(a, dtype=np.float32))
    u = a.view(np.uint32)
    lsb = (u >> np.uint32(13)) & np.uint32(1)
    r = (u + np.uint32(0x0FFF) + lsb) & np.uint32(0xFFFFE000)
    return r.view(np.float32)


def make_in_maps(x, Wq, bq, Wk, bk, Wo, bo):
    WqT = _round_tf32(np.asarray(Wq, dtype=np.float32).T)
    WkT = _round_tf32(np.asarray(Wk, dtype=np.float32).T)
    WoT = _round_tf32(np.asarray(Wo, dtype=np.float32).T)
    bq = np.ascontiguousarray(np.asarray(bq, dtype=np.float32))
    bk = np.ascontiguousarray(np.asarray(bk, dtype=np.float32))
    bo = np.ascontiguousarray(np.asarray(bo, dtype=np.float32))
    x = np.asarray(x, dtype=np.float32)
    return [
        {
            "xT": _round_tf32(x[c].T),
            "WqT": WqT, "WkT": WkT, "WoT": WoT,
            "bq": bq, "bk": bk, "bo": bo,
        }
        for c in range(NCORES)
    ]


def kernel(x, Wq, bq, Wk, bk, Wo, bo):
    nc = build()
    in_maps = make_in_maps(x, Wq, bq, Wk, bk, Wo, bo)
    res = bass_utils.run_bass_kernel_spmd(nc, in_maps, core_ids=list(range(NCORES)))
    return np.stack([res.results[c]["y"] for c in range(NCORES)]).astype(np.float32)


# revision 7
# speedup vs baseline: 1.2315x; 1.0868x over previous
"""Trainium2 Bass kernel for nn_Attention_86638080295542.

Multi-head attention (12 heads, d=64) with the reference's v=k quirk:
    q = x @ Wq.T + bq ; k = x @ Wk.T + bk ; v = k
    out = softmax(q k^T / sqrt(d)) @ v ;  y = out @ Wo.T + bo

Sharding: batch (B=8) data-parallel across the 8 NeuronCores — core c
computes batch element c end-to-end, no collectives.

Per-core dataflow (all "T" tensors keep the contraction dim on SBUF
partitions so every matmul is a natural lhsT.T @ rhs):
  xT[e,s], WqT/WkT/WoT[e_in,e_out] are pre-transposed on the host.
  qT = Wq @ xT   (+bq per-partition)        [768,1024]
  kT = Wk @ xT   (+bk per-partition)        [768,1024]
  vaug[j, jb, h, 0:64] = k natural (PE transpose of kT), col 64 = 1.0
  per head h: pT[j,i] = exp(scale * kT_h^T qT_h)  (no max-subtraction:
     logits are O(1) for this problem, softmax is shift-invariant)
  outT_h[d,i] (+ rowsum in row 64) = vaug^T @ pT, accumulated over j
  normalize: outT_h *= 1/rowsum (rowsum broadcast across the 64
     partitions via gpsimd partition_broadcast)
  y = outT^T @ WoT + bo

Schedule: the attention inner loop is ACT(exp)-paced, so the PE work
for the next pair's projections and this pair's vaug transposes is
split into ~2-matmul "pieces" and one piece is emitted per j-block
inside the attention loop, keeping the PE busy while ACT drains.  Exp
runs on full [128,1024] score tiles (fewer ACT instructions — HW has a
~150ns fixed cost per activation).  PV lags the scores by 3 j-blocks so
the single PV accumulator is free (previous head's normalization done)
before this head's first PV write.

PSUM budget (8 banks): scores ring 2x[128,1024] (4) + proj/trans ring
2x[128,512] (2) + one PV accumulator [65,1024] (2).
"""

from contextlib import ExitStack

import numpy as np

import concourse.bass as bass
import concourse.tile as tile
from concourse import bacc, mybir
from concourse import bass_utils

S = 1024          # sequence length
E = 768           # embed dim
H = 12            # heads
DH = 64           # head dim
P = 128           # partitions
KT = E // P       # 6 k-tiles over embed dim
ST = S // P       # 8 tiles over sequence
NCH = S // 512    # 2 free-dim chunks of 512 over sequence
SCALE = DH ** -0.5
NCORES = 8

F32 = mybir.dt.float32
F32R = mybir.dt.float32r
BF16 = mybir.dt.bfloat16

# rowsum broadcast: 'gpsimd' = nc.gpsimd.partition_broadcast,
# 'dma' = DRAM round-trip with a 0-step partition AP
BCAST = 'gpsimd'


def _emit(nc, tc, ctx, iters=1, variant='full'):
    xT_d = nc.dram_tensor("xT", [E, S], F32R, kind="ExternalInput")
    WqT_d = nc.dram_tensor("WqT", [E, E], F32R, kind="ExternalInput")
    WkT_d = nc.dram_tensor("WkT", [E, E], F32R, kind="ExternalInput")
    WoT_d = nc.dram_tensor("WoT", [E, E], F32R, kind="ExternalInput")
    bq_d = nc.dram_tensor("bq", [E], F32, kind="ExternalInput")
    bk_d = nc.dram_tensor("bk", [E], F32, kind="ExternalInput")
    bo_d = nc.dram_tensor("bo", [E], F32, kind="ExternalInput")
    y_d = nc.dram_tensor("y", [S, E], F32, kind="ExternalOutput")

    Exp = mybir.ActivationFunctionType.Exp

    const = ctx.enter_context(tc.tile_pool(name="const", bufs=1))
    xt_pool = ctx.enter_context(tc.tile_pool(name="xt", bufs=1))
    outt_pool = ctx.enter_context(tc.tile_pool(name="outt", bufs=1))
    wq_pool = ctx.enter_context(tc.tile_pool(name="wq", bufs=1))
    wk_pool = ctx.enter_context(tc.tile_pool(name="wk", bufs=1))
    wo_pool = ctx.enter_context(tc.tile_pool(name="wo", bufs=1))
    vaug_pool = ctx.enter_context(tc.tile_pool(name="vaug", bufs=1))
    qt_pool = ctx.enter_context(tc.tile_pool(name="qt", bufs=3))
    kt_pool = ctx.enter_context(tc.tile_pool(name="kt", bufs=3))
    pt_pool = ctx.enter_context(tc.tile_pool(name="pt", bufs=6))
    ysb_pool = ctx.enter_context(tc.tile_pool(name="ysb", bufs=4))
    rc_pool = ctx.enter_context(tc.tile_pool(name="rc", bufs=2))
    rb_pool = ctx.enter_context(tc.tile_pool(name="rb", bufs=2))
    ps_sc = ctx.enter_context(tc.tile_pool(name="ps_sc", bufs=2, space="PSUM"))
    ps_w = ctx.enter_context(tc.tile_pool(name="ps_w", bufs=2, space="PSUM"))
    ps_pv = ctx.enter_context(tc.tile_pool(name="ps_pv", bufs=1, space="PSUM"))
    if BCAST == 'dma':
        dram_pool = ctx.enter_context(
            tc.tile_pool(name="dram", bufs=4, space="DRAM"))

    # ---- loop-invariant constants (outside the timing loop) ----
    ident_f32 = const.tile([P, P], F32, tag="ident_f32")
    from concourse.masks import make_identity
    make_identity(nc, ident_f32[:])
    identity = const.tile([P, P], F32R, tag="ident")
    nc.vector.tensor_copy(identity[:], ident_f32[:])
    bq_sb = const.tile([P, KT], F32, tag="bq")
    nc.sync.dma_start(bq_sb[:], bq_d.ap().rearrange("(t p) -> p t", p=P))
    bk_sb = const.tile([P, KT], F32, tag="bk")
    nc.sync.dma_start(bk_sb[:], bk_d.ap().rearrange("(t p) -> p t", p=P))
    # bo broadcast to all 128 partitions via a 0-step partition AP (DRAM APs
    # are not partitioned, so a 0-step leading dim is legal here)
    bo_bc = const.tile([P, E], F32, tag="bo")
    bo_ap = bo_d.ap()
    bo_bcast_src = bass.AP(bo_ap.tensor, bo_ap.offset, [[0, P], [1, E]])
    nc.sync.dma_start(bo_bc[:], bo_bcast_src)

    if iters > 1:
        ctx.enter_context(tc.For_i(0, iters, 1))

    # ---- input loads: xT first (needed in full by proj 0), Wq/Wk in
    # hp-column slices so proj hp only waits for slice hp, WoT last ----
    xT_sb = xt_pool.tile([P, KT, S], F32R, tag="xt")
    WqT_sb = wq_pool.tile([P, KT, E], F32R, tag="wq")
    WkT_sb = wk_pool.tile([P, KT, E], F32R, tag="wk")
    WoT_sb = wo_pool.tile([P, KT, E], F32R, tag="wo")
    xT_r = xT_d.ap().rearrange("(t p) s -> p t s", p=P)
    WqT_r = WqT_d.ap().rearrange("(t p) e -> p t e", p=P)
    WkT_r = WkT_d.ap().rearrange("(t p) e -> p t e", p=P)
    WoT_r = WoT_d.ap().rearrange("(t p) e -> p t e", p=P)
    nc.sync.dma_start(xT_sb[:, 0, :], xT_r[:, 0, :])
    nc.sync.dma_start(WqT_sb[:, :, 0:P], WqT_r[:, :, 0:P])
    nc.sync.dma_start(WkT_sb[:, :, 0:P], WkT_r[:, :, 0:P])
    for t in range(1, KT):
        nc.sync.dma_start(xT_sb[:, t, :], xT_r[:, t, :])
    for hp in range(1, KT):
        c0, c1 = P * hp, P * hp + P
        nc.sync.dma_start(WqT_sb[:, :, c0:c1], WqT_r[:, :, c0:c1])
        nc.sync.dma_start(WkT_sb[:, :, c0:c1], WkT_r[:, :, c0:c1])
    for t in range(KT):
        nc.sync.dma_start(WoT_sb[:, t, :], WoT_r[:, t, :])

    vaug = vaug_pool.tile([P, ST, H, DH + 1], BF16, tag="vaug")
    for jb in range(ST):
        nc.vector.memset(vaug[:, jb, :, DH:DH + 1], 1.0)
    outT_sb = outt_pool.tile([P, KT, S], F32R, tag="outt")

    qps = [None] * KT
    kps = [None] * KT

    def proj_pieces(hp, which, c):
        """One projection chunk as 3 pieces of 2 accumulating mms each; the
        last piece evicts to SBUF with the bias add (DVE)."""
        W_sb, b_sb = (WqT_sb, bq_sb) if which == 'q' else (WkT_sb, bk_sb)
        out_sb = qps[hp] if which == 'q' else kps[hp]
        st = {}

        def piece(tp, first, last):
            def go():
                if first:
                    st['ps'] = ps_w.tile([P, 512], F32, tag="ps_w",
                                         name=f"pj_{which}{hp}_{c}")
                ps = st['ps']
                for t in tp:
                    nc.tensor.matmul(
                        ps[:],
                        W_sb[:, t, 128 * hp:128 * hp + 128],
                        xT_sb[:, t, 512 * c:512 * c + 512],
                        start=(t == 0), stop=(t == KT - 1),
                    )
                if last:
                    nc.vector.tensor_scalar_add(
                        out_sb[:, 512 * c:512 * c + 512], ps[:],
                        b_sb[:, hp:hp + 1])
            return go

        return [piece((0, 1), True, False), piece((2, 3), False, False),
                piece((4, 5), False, True)]

    def trans_pieces(hp, g):
        """4 PE transposes of kp(hp) block g + DVE copy into vaug, as 2
        pieces of 2 transposes each."""
        st = {}

        def piece(j4s, first, last):
            def go():
                if first:
                    st['ps'] = ps_w.tile([P, 512], F32R, tag="ps_w",
                                         name=f"tr_{hp}_{g}")
                ps = st['ps']
                kp = kps[hp]
                for j4 in j4s:
                    jb = 4 * g + j4
                    nc.tensor.transpose(
                        ps[:, 128 * j4:128 * j4 + 128],
                        kp[:, 128 * jb:128 * jb + 128],
                        identity[:],
                    )
                if last:
                    nc.vector.tensor_copy(
                        vaug[:, 4 * g:4 * g + 4, 2 * hp:2 * hp + 2, 0:DH],
                        ps[:].rearrange("p (a b c) -> p a b c", a=4, b=2, c=DH),
                    )
            return go

        return [piece((0, 1), True, False), piece((2, 3), False, True)]

    def alloc_qk(hp):
        qps[hp] = qt_pool.tile([P, S], F32R, tag="qt", name=f"qp_{hp}")
        kps[hp] = kt_pool.tile([P, S], F32R, tag="kt", name=f"kp_{hp}")

    def head_attn(hp, h, pre, fill):
        """Attention for head h of pair hp. `pre` pieces run before the
        j-loop; `fill` pieces are consumed one per j-block inside it."""
        po = DH * (h % 2)
        qp, kp = qps[hp], kps[hp]
        pv = ps_pv.tile([DH + 1, S], F32, tag="ps_pv", name=f"pv_{h}")
        for f in pre:
            f()

        def pv_mms(jb, pt):
            for c in range(NCH):
                nc.tensor.matmul(
                    pv[:, 512 * c:512 * c + 512],
                    vaug[:, jb, h, :],
                    pt[:, 512 * c:512 * c + 512],
                    start=(jb == 0), stop=(jb == ST - 1),
                )

        LAG = 3
        fi = 0
        pts = []
        for jb in range(ST):
            pt = pt_pool.tile([P, S], BF16, tag="pt")
            sps = ps_sc.tile([P, S], F32, tag="ps_sc", name=f"sps_{h}_{jb}")
            for c in range(NCH):
                nc.tensor.matmul(
                    sps[:, 512 * c:512 * c + 512],
                    kp[po:po + DH, 128 * jb:128 * jb + 128],
                    qp[po:po + DH, 512 * c:512 * c + 512],
                    start=True, stop=True,
                )
            nc.scalar.activation(pt[:], sps[:], Exp, scale=SCALE)
            pts.append(pt)
            if fi < len(fill):
                fill[fi]()
                fi += 1
            if jb >= LAG:
                pv_mms(jb - LAG, pts[jb - LAG])
        for jb in range(ST - LAG, ST):
            pv_mms(jb, pts[jb])
        for f in fill[fi:]:
            f()

        # normalization, chunked so the chain latency is ~half a tile:
        # rc = 1/rowsum (DVE, straight off PSUM), broadcast across the 64
        # head-dim partitions on gpsimd, multiply into outT (DVE).
        rc = rc_pool.tile([1, S], F32, tag="rc", name=f"rc_{h}")
        rb = rb_pool.tile([DH, S], F32, tag="rb", name=f"rb_{h}")
        if BCAST == 'dma':
            rd = dram_pool.tile([1, S], F32, tag="rd", name=f"rd_{h}")
        for c in range(NCH):
            cs = slice(512 * c, 512 * c + 512)
            nc.vector.reciprocal(rc[:, cs], pv[DH:DH + 1, cs])
            if BCAST == 'gpsimd':
                nc.gpsimd.partition_broadcast(rb[:, cs], rc[:, cs])
            else:
                nc.sync.dma_start(rd[:, cs], rc[:, cs])
                rd_ap = rd[:, cs]
                nc.sync.dma_start(
                    rb[:, cs],
                    bass.AP(rd_ap.tensor, rd_ap.offset, [[0, DH], [1, 512]]))
        for c in range(NCH):
            cs = slice(512 * c, 512 * c + 512)
            nc.vector.tensor_mul(
                outT_sb[po:po + DH, hp, cs], pv[0:DH, cs], rb[:, cs])

    # ---- pair 0 projections (no attention to hide them under) ----
    alloc_qk(0)
    for pc in (proj_pieces(0, 'q', 0) + proj_pieces(0, 'q', 1)
               + proj_pieces(0, 'k', 0) + proj_pieces(0, 'k', 1)
               + trans_pieces(0, 0) + trans_pieces(0, 1)):
        pc()

    # ---- pairs: attention with next pair's projections as in-loop filler ----
    for hp in range(KT):
        tg0 = trans_pieces(hp, 0) if hp > 0 else []
        tg1 = trans_pieces(hp, 1) if hp > 0 else []
        if hp + 1 < KT:
            alloc_qk(hp + 1)
            q0 = proj_pieces(hp + 1, 'q', 0)
            q1 = proj_pieces(hp + 1, 'q', 1)
            k0 = proj_pieces(hp + 1, 'k', 0)
            k1 = proj_pieces(hp + 1, 'k', 1)
        else:
            q0 = q1 = k0 = k1 = []
        head_attn(hp, 2 * hp, tg0, tg1 + q0 + q1)
        head_attn(hp, 2 * hp + 1, [], k0 + k1)

    # ---- output projection: y = outT^T @ WoT + bo ----
    y_r = y_d.ap().rearrange("(st p) e -> st p e", p=P)
    for st in range(ST):
        ysb = ysb_pool.tile([P, E], F32, tag="ysb")
        for n0 in (0, 384):
            yps = ps_sc.tile([P, 512], F32, tag="ps_sc", name=f"yp_{st}_{n0}")
            for t in range(KT):
                nc.tensor.matmul(
                    yps[:, 0:384],
                    outT_sb[:, t, 128 * st:128 * st + 128],
                    WoT_sb[:, t, n0:n0 + 384],
                    start=(t == 0), stop=(t == KT - 1),
                )
            nc.vector.tensor_add(ysb[:, n0:n0 + 384], yps[:, 0:384],
                                 bo_bc[:, n0:n0 + 384])
        # stores ride the ACT hwdge queue so next iteration's input loads
        # on the sync queue are not serialized behind them
        nc.scalar.dma_start(y_r[st], ysb[:])


_NC_CACHE = {}


def build(iters=1, variant="full"):
    key = (iters, variant)
    nc = _NC_CACHE.get(key)
    if nc is None:
        nc = bacc.Bacc("TRN2", target_bir_lowering=False, debug=False)
        with tile.TileContext(nc) as tc, ExitStack() as ctx:
            _emit(nc, tc, ctx, iters=iters, variant=variant)
        nc.compile()
        _NC_CACHE[key] = nc
    return nc


def _round_tf32(a):
    """Round fp32 to tf32 (10 explicit mantissa bits), RNE, fp32 container."""
    a = np.ascontiguousarray(np.asarray(a, dtype=np.float32))
    u = a.view(np.uint32)
    lsb = (u >> np.uint32(13)) & np.uint32(1)
    r = (u + np.uint32(0x0FFF) + lsb) & np.uint32(0xFFFFE000)
    return r.view(np.float32)


def make_in_maps(x, Wq, bq, Wk, bk, Wo, bo):
    WqT = _round_tf32(np.asarray(Wq, dtype=np.float32).T)
    WkT = _round_tf32(np.asarray(Wk, dtype=np.float32).T)
    WoT = _round_tf32(np.asarray(Wo, dtype=np.float32).T)
    bq = np.ascontiguousarray(np.asarray(bq, dtype=np.float32))
    bk = np.ascontiguousarray(np.asarray(bk, dtype=np.float32))
    bo = np.ascontiguousarray(np.asarray(bo, dtype=np.float32))
    x = np.asarray(x, dtype=np.float32)
    return [
        {
            "xT": _round_tf32(x[c].T),
            "WqT": WqT, "WkT": WkT, "WoT": WoT,
            "bq": bq, "bk": bk, "bo": bo,
        }
        for c in range(NCORES)
    ]


def kernel(x, Wq, bq, Wk, bk, Wo, bo):
    nc = build()
    in_maps = make_in_maps(x, Wq, bq, Wk, bk, Wo, bo)
    res = bass_utils.run_bass_kernel_spmd(nc, in_maps, core_ids=list(range(NCORES)))
    return np.stack([res.results[c]["y"] for c in range(NCORES)]).astype(np.float32)


# revision 14
# speedup vs baseline: 1.4172x; 1.1508x over previous
"""Trainium2 Bass kernel for nn_Attention_86638080295542.

Multi-head attention (12 heads, d=64) with the reference's v=k quirk:
    q = x @ Wq.T + bq ; k = x @ Wk.T + bk ; v = k
    out = softmax(q k^T / sqrt(d)) @ v ;  y = out @ Wo.T + bo

Sharding: batch (B=8) data-parallel across the 8 NeuronCores — core c
computes batch element c end-to-end, no collectives.

Per-core dataflow (all "T" tensors keep the contraction dim on SBUF
partitions so every matmul is a natural lhsT.T @ rhs):
  xT[e,s], WqT/WkT/WoT[e_in,e_out] are pre-transposed on the host.
  qT = Wq @ xT   (+bq per-partition)        [768,1024]
  kT = Wk @ xT   (+bk per-partition)        [768,1024]
  vaug[j, jb, h, 0:64] = k natural (PE transpose of kT), col 64 = 1.0
  per head h: pT[j,i] = exp(scale * kT_h^T qT_h)  (no max-subtraction:
     logits are O(1) for this problem, softmax is shift-invariant)
  outT_h[d,i] (+ rowsum in row 64) = vaug^T @ pT, accumulated over j
  normalize: outT_h *= 1/rowsum (rowsum broadcast across the 64
     partitions via gpsimd partition_broadcast)
  y = outT^T @ WoT + bo

Schedule: the attention inner loop is ACT(exp)-paced, so the PE work
for the next pair's projections and this pair's vaug transposes is
split into ~2-matmul "pieces" and one piece is emitted per j-block
inside the attention loop, keeping the PE busy while ACT drains.  Exp
runs on full [128,1024] score tiles (fewer ACT instructions — HW has a
~150ns fixed cost per activation).  PV lags the scores by 3 j-blocks so
the single PV accumulator is free (previous head's normalization done)
before this head's first PV write.

PSUM budget (8 banks): scores ring 2x[128,1024] (4) + proj/trans ring
2x[128,512] (2) + one PV accumulator [65,1024] (2).
"""

from contextlib import ExitStack

import numpy as np

import concourse.bass as bass
import concourse.tile as tile
from concourse import bacc, mybir
from concourse import bass_utils

S = 1024          # sequence length
E = 768           # embed dim
H = 12            # heads
DH = 64           # head dim
P = 128           # partitions
KT = E // P       # 6 k-tiles over embed dim
ST = S // P       # 8 tiles over sequence
NCH = S // 512    # 2 free-dim chunks of 512 over sequence
SCALE = DH ** -0.5
NCORES = 8

F32 = mybir.dt.float32
F32R = mybir.dt.float32r
BF16 = mybir.dt.bfloat16

# rowsum broadcast: 'gpsimd' = nc.gpsimd.partition_broadcast,
# 'dma' = DRAM round-trip with a 0-step partition AP
BCAST = 'gpsimd'


def _emit(nc, tc, ctx, iters=1, variant='full'):
    xT_d = nc.dram_tensor("xT", [E, S], BF16, kind="ExternalInput")
    WqT_d = nc.dram_tensor("WqT", [E, E], BF16, kind="ExternalInput")
    WkT_d = nc.dram_tensor("WkT", [E, E], BF16, kind="ExternalInput")
    WoT_d = nc.dram_tensor("WoT", [E, E], BF16, kind="ExternalInput")
    bq_d = nc.dram_tensor("bq", [E], F32, kind="ExternalInput")
    bk_d = nc.dram_tensor("bk", [E], F32, kind="ExternalInput")
    bo_d = nc.dram_tensor("bo", [E], F32, kind="ExternalInput")
    y_d = nc.dram_tensor("y", [S, E], F32, kind="ExternalOutput")

    Exp = mybir.ActivationFunctionType.Exp

    const = ctx.enter_context(tc.tile_pool(name="const", bufs=1))
    xt_pool = ctx.enter_context(tc.tile_pool(name="xt", bufs=1))
    outt_pool = ctx.enter_context(tc.tile_pool(name="outt", bufs=1))
    wq_pool = ctx.enter_context(tc.tile_pool(name="wq", bufs=1))
    wk_pool = ctx.enter_context(tc.tile_pool(name="wk", bufs=1))
    wo_pool = ctx.enter_context(tc.tile_pool(name="wo", bufs=1))
    vaug_pool = ctx.enter_context(tc.tile_pool(name="vaug", bufs=1))
    qt_pool = ctx.enter_context(tc.tile_pool(name="qt", bufs=3))
    kt_pool = ctx.enter_context(tc.tile_pool(name="kt", bufs=3))
    pt_pool = ctx.enter_context(tc.tile_pool(name="pt", bufs=6))
    ysb_pool = ctx.enter_context(tc.tile_pool(name="ysb", bufs=4))
    rc_pool = ctx.enter_context(tc.tile_pool(name="rc", bufs=2))
    rb_pool = ctx.enter_context(tc.tile_pool(name="rb", bufs=2))
    ps_sc = ctx.enter_context(tc.tile_pool(name="ps_sc", bufs=2, space="PSUM"))
    ps_w = ctx.enter_context(tc.tile_pool(name="ps_w", bufs=2, space="PSUM"))
    ps_pv = ctx.enter_context(tc.tile_pool(name="ps_pv", bufs=1, space="PSUM"))
    if BCAST == 'dma':
        dram_pool = ctx.enter_context(
            tc.tile_pool(name="dram", bufs=4, space="DRAM"))

    # ---- loop-invariant constants (outside the timing loop) ----
    ident_f32 = const.tile([P, P], F32, tag="ident_f32")
    from concourse.masks import make_identity
    make_identity(nc, ident_f32[:])
    identity = const.tile([P, P], BF16, tag="ident")
    nc.vector.tensor_copy(identity[:], ident_f32[:])
    bq_sb = const.tile([P, KT], F32, tag="bq")
    nc.sync.dma_start(bq_sb[:], bq_d.ap().rearrange("(t p) -> p t", p=P))
    bk_sb = const.tile([P, KT], F32, tag="bk")
    nc.sync.dma_start(bk_sb[:], bk_d.ap().rearrange("(t p) -> p t", p=P))
    # bo broadcast to all 128 partitions via a 0-step partition AP (DRAM APs
    # are not partitioned, so a 0-step leading dim is legal here)
    bo_bc = const.tile([P, E], F32, tag="bo")
    bo_ap = bo_d.ap()
    bo_bcast_src = bass.AP(bo_ap.tensor, bo_ap.offset, [[0, P], [1, E]])
    nc.sync.dma_start(bo_bc[:], bo_bcast_src)

    if iters > 1:
        ctx.enter_context(tc.For_i(0, iters, 1))

    # ---- input loads: xT first (needed in full by proj 0), Wq/Wk in
    # hp-column slices so proj hp only waits for slice hp, WoT last ----
    xT_sb = xt_pool.tile([P, KT, S], BF16, tag="xt")
    WqT_sb = wq_pool.tile([P, KT, E], BF16, tag="wq")
    WkT_sb = wk_pool.tile([P, KT, E], BF16, tag="wk")
    WoT_sb = wo_pool.tile([P, KT, E], BF16, tag="wo")
    xT_r = xT_d.ap().rearrange("(t p) s -> p t s", p=P)
    WqT_r = WqT_d.ap().rearrange("(t p) e -> p t e", p=P)
    WkT_r = WkT_d.ap().rearrange("(t p) e -> p t e", p=P)
    WoT_r = WoT_d.ap().rearrange("(t p) e -> p t e", p=P)
    nc.sync.dma_start(xT_sb[:, 0, :], xT_r[:, 0, :])
    nc.sync.dma_start(WqT_sb[:, :, 0:P], WqT_r[:, :, 0:P])
    nc.sync.dma_start(WkT_sb[:, :, 0:P], WkT_r[:, :, 0:P])
    for t in range(1, KT):
        nc.sync.dma_start(xT_sb[:, t, :], xT_r[:, t, :])
    for hp in range(1, KT):
        c0, c1 = P * hp, P * hp + P
        nc.sync.dma_start(WqT_sb[:, :, c0:c1], WqT_r[:, :, c0:c1])
        nc.sync.dma_start(WkT_sb[:, :, c0:c1], WkT_r[:, :, c0:c1])
    for t in range(KT):
        nc.sync.dma_start(WoT_sb[:, t, :], WoT_r[:, t, :])

    vaug = vaug_pool.tile([P, ST, H, DH + 1], BF16, tag="vaug")
    for jb in range(ST):
        nc.vector.memset(vaug[:, jb, :, DH:DH + 1], 1.0)
    outT_sb = outt_pool.tile([P, KT, S], BF16, tag="outt")

    if variant == 'noexp':
        pt_const = const.tile([P, S], BF16, tag="ptc")
        nc.vector.memset(pt_const[:], 0.01)
        scr = const.tile([P, 256], F32, tag="scr")

    qps = [None] * KT
    kps = [None] * KT

    def proj_pieces(hp, which, c):
        """One projection chunk as 3 pieces of 2 accumulating mms each; the
        last piece evicts to SBUF with the bias add (DVE)."""
        W_sb, b_sb = (WqT_sb, bq_sb) if which == 'q' else (WkT_sb, bk_sb)
        out_sb = qps[hp] if which == 'q' else kps[hp]
        st = {}

        def piece(tp, first, last):
            def go():
                if first:
                    st['ps'] = ps_w.tile([P, 512], F32, tag="ps_w",
                                         name=f"pj_{which}{hp}_{c}")
                ps = st['ps']
                for t in tp:
                    nc.tensor.matmul(
                        ps[:],
                        W_sb[:, t, 128 * hp:128 * hp + 128],
                        xT_sb[:, t, 512 * c:512 * c + 512],
                        start=(t == 0), stop=(t == KT - 1),
                    )
                if last:
                    nc.vector.tensor_scalar_add(
                        out_sb[:, 512 * c:512 * c + 512], ps[:],
                        b_sb[:, hp:hp + 1])
            return go

        return [piece((0, 1), True, False), piece((2, 3), False, False),
                piece((4, 5), False, True)]

    def trans_pieces(hp, g):
        """4 PE transposes of kp(hp) block g + DVE copy into vaug, as 2
        pieces of 2 transposes each."""
        st = {}

        def piece(j4s, first, last):
            def go():
                if first:
                    st['ps'] = ps_w.tile([P, 512], BF16, tag="ps_w",
                                         name=f"tr_{hp}_{g}")
                ps = st['ps']
                kp = kps[hp]
                for j4 in j4s:
                    jb = 4 * g + j4
                    nc.tensor.transpose(
                        ps[:, 128 * j4:128 * j4 + 128],
                        kp[:, 128 * jb:128 * jb + 128],
                        identity[:],
                    )
                if last:
                    nc.vector.tensor_copy(
                        vaug[:, 4 * g:4 * g + 4, 2 * hp:2 * hp + 2, 0:DH],
                        ps[:].rearrange("p (a b c) -> p a b c", a=4, b=2, c=DH),
                    )
            return go

        return [piece((0, 1), True, False), piece((2, 3), False, True)]

    def alloc_qk(hp):
        qps[hp] = qt_pool.tile([P, S], BF16, tag="qt", name=f"qp_{hp}")
        kps[hp] = kt_pool.tile([P, S], BF16, tag="kt", name=f"kp_{hp}")

    def head_attn(hp, h, pre, fill):
        """Attention for head h of pair hp. `pre` pieces run before the
        j-loop; `fill` pieces are consumed one per j-block inside it."""
        po = DH * (h % 2)
        if variant == 'noattn':
            for f in pre:
                f()
            for f in fill:
                f()
            if h % 2 == 0:
                nc.vector.memset(outT_sb[:, hp, :], 0.01)
            return None
        qp, kp = qps[hp], kps[hp]
        pv = ps_pv.tile([DH + 1, S], F32, tag="ps_pv", name=f"pv_{h}")
        for f in pre:
            f()

        def pv_mms(jb, pt):
            for c in range(NCH):
                nc.tensor.matmul(
                    pv[:, 512 * c:512 * c + 512],
                    vaug[:, jb, h, :],
                    pt[:, 512 * c:512 * c + 512],
                    start=(jb == 0), stop=(jb == ST - 1),
                )

        LAG = 3
        fi = 0
        pts = []
        for jb in range(ST):
            pt = pt_pool.tile([P, S], BF16, tag="pt")
            sps = ps_sc.tile([P, S], F32, tag="ps_sc", name=f"sps_{h}_{jb}")
            for c in range(NCH):
                nc.tensor.matmul(
                    sps[:, 512 * c:512 * c + 512],
                    kp[po:po + DH, 128 * jb:128 * jb + 128],
                    qp[po:po + DH, 512 * c:512 * c + 512],
                    start=True, stop=True,
                )
            if variant == 'noexp':
                # skeleton timing: keep scores mms (cheap reader frees the
                # ring slot), drop the ACT dependency from the PV chain
                nc.vector.tensor_copy(scr[:, 2 * (jb % 8):2 * (jb % 8) + 1],
                                      sps[:, 0:1])
                pt = pt_const
            else:
                nc.scalar.activation(pt[:], sps[:], Exp, scale=SCALE)
            pts.append(pt)
            if fi < len(fill):
                fill[fi]()
                fi += 1
            if jb >= LAG:
                pv_mms(jb - LAG, pts[jb - LAG])
        for jb in range(ST - LAG, ST):
            pv_mms(jb, pts[jb])
        for f in fill[fi:]:
            f()

        # normalization, chunked so the chain latency is ~half a tile:
        # rc = 1/rowsum (DVE, straight off PSUM), broadcast across the 64
        # head-dim partitions on gpsimd.  The multiplies are returned as a
        # closure and emitted early in the NEXT head's fill schedule, so
        # their wait on the broadcast does not head-of-line-block the DVE
        # FIFO behind them (psum-freeing evictions).
        rc = rc_pool.tile([1, S], F32, tag="rc", name=f"rc_{h}")
        rb = rb_pool.tile([DH, S], F32, tag="rb", name=f"rb_{h}")
        if BCAST == 'dma':
            rd = dram_pool.tile([1, S], F32, tag="rd", name=f"rd_{h}")
        for c in range(NCH):
            cs = slice(512 * c, 512 * c + 512)
            nc.vector.reciprocal(rc[:, cs], pv[DH:DH + 1, cs])
            if BCAST == 'gpsimd':
                nc.gpsimd.partition_broadcast(rb[:, cs], rc[:, cs])
            else:
                nc.sync.dma_start(rd[:, cs], rc[:, cs])
                rd_ap = rd[:, cs]
                nc.sync.dma_start(
                    rb[:, cs],
                    bass.AP(rd_ap.tensor, rd_ap.offset, [[0, DH], [1, 512]]))

        def finish_norm():
            for c in range(NCH):
                cs = slice(512 * c, 512 * c + 512)
                nc.vector.tensor_mul(
                    outT_sb[po:po + DH, hp, cs], pv[0:DH, cs], rb[:, cs])
        return finish_norm

    # ---- pair 0 projections (no attention to hide them under) ----
    alloc_qk(0)
    for pc in (proj_pieces(0, 'q', 0) + proj_pieces(0, 'q', 1)
               + proj_pieces(0, 'k', 0) + proj_pieces(0, 'k', 1)
               + trans_pieces(0, 0) + trans_pieces(0, 1)):
        pc()

    # ---- pairs: attention with next pair's projections as in-loop filler ----
    pending = None
    for hp in range(KT):
        tg0 = trans_pieces(hp, 0) if hp > 0 else []
        tg1 = trans_pieces(hp, 1) if hp > 0 else []
        if hp + 1 < KT:
            alloc_qk(hp + 1)
            q0 = proj_pieces(hp + 1, 'q', 0)
            q1 = proj_pieces(hp + 1, 'q', 1)
            k0 = proj_pieces(hp + 1, 'k', 0)
            k1 = proj_pieces(hp + 1, 'k', 1)
        else:
            q0 = q1 = k0 = k1 = []
        pA = [pending] if pending else []
        fA = head_attn(hp, 2 * hp, tg0, pA + tg1 + q0 + q1)
        pB = [fA] if fA else []
        pending = head_attn(hp, 2 * hp + 1, [], pB + k0 + k1)
    if pending:
        pending()

    # ---- output projection: y = outT^T @ WoT + bo ----
    y_r = y_d.ap().rearrange("(st p) e -> st p e", p=P)
    for st in range(ST):
        ysb = ysb_pool.tile([P, E], F32, tag="ysb")
        for n0 in (0, 384):
            yps = ps_sc.tile([P, 512], F32, tag="ps_sc", name=f"yp_{st}_{n0}")
            for t in range(KT):
                nc.tensor.matmul(
                    yps[:, 0:384],
                    outT_sb[:, t, 128 * st:128 * st + 128],
                    WoT_sb[:, t, n0:n0 + 384],
                    start=(t == 0), stop=(t == KT - 1),
                )
            nc.vector.tensor_add(ysb[:, n0:n0 + 384], yps[:, 0:384],
                                 bo_bc[:, n0:n0 + 384])
        # stores ride the ACT hwdge queue so next iteration's input loads
        # on the sync queue are not serialized behind them
        nc.scalar.dma_start(y_r[st], ysb[:])


_NC_CACHE = {}


def build(iters=1, variant="full"):
    key = (iters, variant)
    nc = _NC_CACHE.get(key)
    if nc is None:
        nc = bacc.Bacc("TRN2", target_bir_lowering=False, debug=False)
        with tile.TileContext(nc) as tc, ExitStack() as ctx:
            _emit(nc, tc, ctx, iters=iters, variant=variant)
        nc.compile()
        _NC_CACHE[key] = nc
    return nc


def _round_tf32(a):
    """Round fp32 to tf32 (10 explicit mantissa bits), RNE, fp32 container."""
    a = np.ascontiguousarray(np.asarray(a, dtype=np.float32))
    u = a.view(np.uint32)
    lsb = (u >> np.uint32(13)) & np.uint32(1)
    r = (u + np.uint32(0x0FFF) + lsb) & np.uint32(0xFFFFE000)
    return r.view(np.float32)


def make_in_maps(x, Wq, bq, Wk, bk, Wo, bo):
    import ml_dtypes
    BF = ml_dtypes.bfloat16
    WqT = np.ascontiguousarray(np.asarray(Wq, dtype=np.float32).T).astype(BF)
    WkT = np.ascontiguousarray(np.asarray(Wk, dtype=np.float32).T).astype(BF)
    WoT = np.ascontiguousarray(np.asarray(Wo, dtype=np.float32).T).astype(BF)
    bq = np.ascontiguousarray(np.asarray(bq, dtype=np.float32))
    bk = np.ascontiguousarray(np.asarray(bk, dtype=np.float32))
    bo = np.ascontiguousarray(np.asarray(bo, dtype=np.float32))
    x = np.asarray(x, dtype=np.float32)
    return [
        {
            "xT": np.ascontiguousarray(x[c].T).astype(BF),
            "WqT": WqT, "WkT": WkT, "WoT": WoT,
            "bq": bq, "bk": bk, "bo": bo,
        }
        for c in range(NCORES)
    ]


def kernel(x, Wq, bq, Wk, bk, Wo, bo):
    nc = build()
    in_maps = make_in_maps(x, Wq, bq, Wk, bk, Wo, bo)
    res = bass_utils.run_bass_kernel_spmd(nc, in_maps, core_ids=list(range(NCORES)))
    return np.stack([res.results[c]["y"] for c in range(NCORES)]).astype(np.float32)


# revision 15
# speedup vs baseline: 1.4345x; 1.0123x over previous
"""Trainium2 Bass kernel for nn_Attention_86638080295542.

Multi-head attention (12 heads, d=64) with the reference's v=k quirk:
    q = x @ Wq.T + bq ; k = x @ Wk.T + bk ; v = k
    out = softmax(q k^T / sqrt(d)) @ v ;  y = out @ Wo.T + bo

Sharding: batch (B=8) data-parallel across the 8 NeuronCores — core c
computes batch element c end-to-end, no collectives.

Per-core dataflow (all "T" tensors keep the contraction dim on SBUF
partitions so every matmul is a natural lhsT.T @ rhs):
  xT[e,s], WqT/WkT/WoT[e_in,e_out] are pre-transposed on the host.
  qT = Wq @ xT   (+bq per-partition)        [768,1024]
  kT = Wk @ xT   (+bk per-partition)        [768,1024]
  vaug[j, jb, h, 0:64] = k natural (PE transpose of kT), col 64 = 1.0
  per head h: pT[j,i] = exp(scale * kT_h^T qT_h)  (no max-subtraction:
     logits are O(1) for this problem, softmax is shift-invariant)
  outT_h[d,i] (+ rowsum in row 64) = vaug^T @ pT, accumulated over j
  normalize: outT_h *= 1/rowsum (rowsum broadcast across the 64
     partitions via gpsimd partition_broadcast)
  y = outT^T @ WoT + bo

Schedule: the attention inner loop is ACT(exp)-paced, so the PE work
for the next pair's projections and this pair's vaug transposes is
split into ~2-matmul "pieces" and one piece is emitted per j-block
inside the attention loop, keeping the PE busy while ACT drains.  Exp
runs on full [128,1024] score tiles (fewer ACT instructions — HW has a
~150ns fixed cost per activation).  PV lags the scores by 3 j-blocks so
the single PV accumulator is free (previous head's normalization done)
before this head's first PV write.

PSUM budget (8 banks): scores ring 2x[128,1024] (4) + proj/trans ring
2x[128,512] (2) + one PV accumulator [65,1024] (2).
"""

from contextlib import ExitStack

import numpy as np

import concourse.bass as bass
import concourse.tile as tile
from concourse import bacc, mybir
from concourse import bass_utils

S = 1024          # sequence length
E = 768           # embed dim
H = 12            # heads
DH = 64           # head dim
P = 128           # partitions
KT = E // P       # 6 k-tiles over embed dim
ST = S // P       # 8 tiles over sequence
NCH = S // 512    # 2 free-dim chunks of 512 over sequence
SCALE = DH ** -0.5
NCORES = 8

F32 = mybir.dt.float32
BF16 = mybir.dt.bfloat16

# rowsum broadcast: 'gpsimd' = nc.gpsimd.partition_broadcast,
# 'dma' = DRAM round-trip with a 0-step partition AP
BCAST = 'gpsimd'


def _emit(nc, tc, ctx, iters=1, variant='full'):
    xT_d = nc.dram_tensor("xT", [E, S], BF16, kind="ExternalInput")
    WqT_d = nc.dram_tensor("WqT", [E, E], BF16, kind="ExternalInput")
    WkT_d = nc.dram_tensor("WkT", [E, E], BF16, kind="ExternalInput")
    WoT_d = nc.dram_tensor("WoT", [E, E], BF16, kind="ExternalInput")
    bq_d = nc.dram_tensor("bq", [E], F32, kind="ExternalInput")
    bk_d = nc.dram_tensor("bk", [E], F32, kind="ExternalInput")
    bo_d = nc.dram_tensor("bo", [E], F32, kind="ExternalInput")
    y_d = nc.dram_tensor("y", [S, E], F32, kind="ExternalOutput")

    Exp = mybir.ActivationFunctionType.Exp

    const = ctx.enter_context(tc.tile_pool(name="const", bufs=1))
    xt_pool = ctx.enter_context(tc.tile_pool(name="xt", bufs=1))
    outt_pool = ctx.enter_context(tc.tile_pool(name="outt", bufs=1))
    wq_pool = ctx.enter_context(tc.tile_pool(name="wq", bufs=1))
    wk_pool = ctx.enter_context(tc.tile_pool(name="wk", bufs=1))
    wo_pool = ctx.enter_context(tc.tile_pool(name="wo", bufs=1))
    vaug_pool = ctx.enter_context(tc.tile_pool(name="vaug", bufs=1))
    qt_pool = ctx.enter_context(tc.tile_pool(name="qt", bufs=3))
    kt_pool = ctx.enter_context(tc.tile_pool(name="kt", bufs=3))
    pt_pool = ctx.enter_context(tc.tile_pool(name="pt", bufs=6))
    ysb_pool = ctx.enter_context(tc.tile_pool(name="ysb", bufs=4))
    rc_pool = ctx.enter_context(tc.tile_pool(name="rc", bufs=2))
    rb_pool = ctx.enter_context(tc.tile_pool(name="rb", bufs=2))
    ps_sc = ctx.enter_context(tc.tile_pool(name="ps_sc", bufs=2, space="PSUM"))
    ps_w = ctx.enter_context(tc.tile_pool(name="ps_w", bufs=2, space="PSUM"))
    ps_pv = ctx.enter_context(tc.tile_pool(name="ps_pv", bufs=1, space="PSUM"))
    if BCAST == 'dma':
        dram_pool = ctx.enter_context(
            tc.tile_pool(name="dram", bufs=4, space="DRAM"))

    # ---- loop-invariant constants (outside the timing loop) ----
    ident_f32 = const.tile([P, P], F32, tag="ident_f32")
    from concourse.masks import make_identity
    make_identity(nc, ident_f32[:])
    identity = const.tile([P, P], BF16, tag="ident")
    nc.vector.tensor_copy(identity[:], ident_f32[:])
    bq_sb = const.tile([P, KT], F32, tag="bq")
    nc.sync.dma_start(bq_sb[:], bq_d.ap().rearrange("(t p) -> p t", p=P))
    bk_sb = const.tile([P, KT], F32, tag="bk")
    nc.sync.dma_start(bk_sb[:], bk_d.ap().rearrange("(t p) -> p t", p=P))
    # bo broadcast to all 128 partitions via a 0-step partition AP (DRAM APs
    # are not partitioned, so a 0-step leading dim is legal here)
    bo_bc = const.tile([P, E], F32, tag="bo")
    bo_ap = bo_d.ap()
    bo_bcast_src = bass.AP(bo_ap.tensor, bo_ap.offset, [[0, P], [1, E]])
    nc.sync.dma_start(bo_bc[:], bo_bcast_src)

    if iters > 1:
        ctx.enter_context(tc.For_i(0, iters, 1))

    # ---- input loads: xT first (needed in full by proj 0), Wq/Wk in
    # hp-column slices so proj hp only waits for slice hp, WoT last ----
    xT_sb = xt_pool.tile([P, KT, S], BF16, tag="xt")
    WqT_sb = wq_pool.tile([P, KT, E], BF16, tag="wq")
    WkT_sb = wk_pool.tile([P, KT, E], BF16, tag="wk")
    WoT_sb = wo_pool.tile([P, KT, E], BF16, tag="wo")
    xT_r = xT_d.ap().rearrange("(t p) s -> p t s", p=P)
    WqT_r = WqT_d.ap().rearrange("(t p) e -> p t e", p=P)
    WkT_r = WkT_d.ap().rearrange("(t p) e -> p t e", p=P)
    WoT_r = WoT_d.ap().rearrange("(t p) e -> p t e", p=P)
    nc.sync.dma_start(xT_sb[:, 0, :], xT_r[:, 0, :])
    nc.sync.dma_start(WqT_sb[:, :, 0:P], WqT_r[:, :, 0:P])
    nc.sync.dma_start(WkT_sb[:, :, 0:P], WkT_r[:, :, 0:P])
    for t in range(1, KT):
        nc.sync.dma_start(xT_sb[:, t, :], xT_r[:, t, :])
    for hp in range(1, KT):
        c0, c1 = P * hp, P * hp + P
        nc.sync.dma_start(WqT_sb[:, :, c0:c1], WqT_r[:, :, c0:c1])
        nc.sync.dma_start(WkT_sb[:, :, c0:c1], WkT_r[:, :, c0:c1])
    for t in range(KT):
        nc.sync.dma_start(WoT_sb[:, t, :], WoT_r[:, t, :])

    vaug = vaug_pool.tile([P, ST, H, DH + 1], BF16, tag="vaug")
    for jb in range(ST):
        nc.vector.memset(vaug[:, jb, :, DH:DH + 1], 1.0)
    outT_sb = outt_pool.tile([P, KT, S], BF16, tag="outt")

    if variant == 'noexp':
        pt_const = const.tile([P, S], BF16, tag="ptc")
        nc.vector.memset(pt_const[:], 0.01)
        scr = const.tile([P, 256], F32, tag="scr")

    qps = [None] * KT
    kps = [None] * KT

    def proj_pieces(hp, which, c):
        """One projection chunk as 3 pieces of 2 accumulating mms each; the
        last piece evicts to SBUF with the bias add (DVE)."""
        W_sb, b_sb = (WqT_sb, bq_sb) if which == 'q' else (WkT_sb, bk_sb)
        out_sb = qps[hp] if which == 'q' else kps[hp]
        st = {}

        def piece(tp, first, last):
            def go():
                if first:
                    st['ps'] = ps_w.tile([P, 512], F32, tag="ps_w",
                                         name=f"pj_{which}{hp}_{c}")
                ps = st['ps']
                for t in tp:
                    nc.tensor.matmul(
                        ps[:],
                        W_sb[:, t, 128 * hp:128 * hp + 128],
                        xT_sb[:, t, 512 * c:512 * c + 512],
                        start=(t == 0), stop=(t == KT - 1),
                    )
                if last:
                    nc.vector.tensor_scalar_add(
                        out_sb[:, 512 * c:512 * c + 512], ps[:],
                        b_sb[:, hp:hp + 1])
            return go

        return [piece((0, 1), True, False), piece((2, 3), False, False),
                piece((4, 5), False, True)]

    def trans_pieces(hp, g):
        """4 PE transposes of kp(hp) block g + DVE copy into vaug, as 2
        pieces of 2 transposes each."""
        st = {}

        def piece(j4s, first, last):
            def go():
                if first:
                    st['ps'] = ps_w.tile([P, 512], BF16, tag="ps_w",
                                         name=f"tr_{hp}_{g}")
                ps = st['ps']
                kp = kps[hp]
                for j4 in j4s:
                    jb = 4 * g + j4
                    nc.tensor.transpose(
                        ps[:, 128 * j4:128 * j4 + 128],
                        kp[:, 128 * jb:128 * jb + 128],
                        identity[:],
                    )
                if last:
                    nc.vector.tensor_copy(
                        vaug[:, 4 * g:4 * g + 4, 2 * hp:2 * hp + 2, 0:DH],
                        ps[:].rearrange("p (a b c) -> p a b c", a=4, b=2, c=DH),
                    )
            return go

        return [piece((0, 1), True, False), piece((2, 3), False, True)]

    def alloc_qk(hp):
        qps[hp] = qt_pool.tile([P, S], BF16, tag="qt", name=f"qp_{hp}")
        kps[hp] = kt_pool.tile([P, S], BF16, tag="kt", name=f"kp_{hp}")

    def head_attn(hp, h, pre, fill):
        """Attention for head h of pair hp. `pre` pieces run before the
        j-loop; `fill` pieces are consumed one per j-block inside it."""
        po = DH * (h % 2)
        if variant == 'noattn':
            for f in pre:
                f()
            for f in fill:
                f()
            if h % 2 == 0:
                nc.vector.memset(outT_sb[:, hp, :], 0.01)
            return None
        qp, kp = qps[hp], kps[hp]
        pv = ps_pv.tile([DH + 1, S], F32, tag="ps_pv", name=f"pv_{h}")
        for f in pre:
            f()

        def pv_mms(jb, pt):
            for c in range(NCH):
                nc.tensor.matmul(
                    pv[:, 512 * c:512 * c + 512],
                    vaug[:, jb, h, :],
                    pt[:, 512 * c:512 * c + 512],
                    start=(jb == 0), stop=(jb == ST - 1),
                )

        LAG = 3
        fi = 0
        pts = []
        for jb in range(ST):
            pt = pt_pool.tile([P, S], BF16, tag="pt")
            sps = ps_sc.tile([P, S], F32, tag="ps_sc", name=f"sps_{h}_{jb}")
            for c in range(NCH):
                nc.tensor.matmul(
                    sps[:, 512 * c:512 * c + 512],
                    kp[po:po + DH, 128 * jb:128 * jb + 128],
                    qp[po:po + DH, 512 * c:512 * c + 512],
                    start=True, stop=True,
                )
            if variant == 'noexp':
                # skeleton timing: keep scores mms (cheap reader frees the
                # ring slot), drop the ACT dependency from the PV chain
                nc.vector.tensor_copy(scr[:, 2 * (jb % 8):2 * (jb % 8) + 1],
                                      sps[:, 0:1])
                pt = pt_const
            else:
                nc.scalar.activation(pt[:], sps[:], Exp, scale=SCALE)
            pts.append(pt)
            if fi < len(fill):
                fill[fi]()
                fi += 1
            if jb >= LAG:
                pv_mms(jb - LAG, pts[jb - LAG])
        for jb in range(ST - LAG, ST):
            pv_mms(jb, pts[jb])
        for f in fill[fi:]:
            f()

        # normalization, chunked so the chain latency is ~half a tile:
        # rc = 1/rowsum (DVE, straight off PSUM), broadcast across the 64
        # head-dim partitions on gpsimd.  The multiplies are returned as a
        # closure and emitted early in the NEXT head's fill schedule, so
        # their wait on the broadcast does not head-of-line-block the DVE
        # FIFO behind them (psum-freeing evictions).
        rc = rc_pool.tile([1, S], F32, tag="rc", name=f"rc_{h}")
        rb = rb_pool.tile([DH, S], F32, tag="rb", name=f"rb_{h}")
        if BCAST == 'dma':
            rd = dram_pool.tile([1, S], F32, tag="rd", name=f"rd_{h}")
        for c in range(NCH):
            cs = slice(512 * c, 512 * c + 512)
            nc.vector.reciprocal(rc[:, cs], pv[DH:DH + 1, cs])
            if BCAST == 'gpsimd':
                nc.gpsimd.partition_broadcast(rb[:, cs], rc[:, cs])
            else:
                nc.sync.dma_start(rd[:, cs], rc[:, cs])
                rd_ap = rd[:, cs]
                nc.sync.dma_start(
                    rb[:, cs],
                    bass.AP(rd_ap.tensor, rd_ap.offset, [[0, DH], [1, 512]]))

        def finish_norm():
            for c in range(NCH):
                cs = slice(512 * c, 512 * c + 512)
                nc.vector.tensor_mul(
                    outT_sb[po:po + DH, hp, cs], pv[0:DH, cs], rb[:, cs])
        return finish_norm

    # ---- pair 0 projections (no attention to hide them under) ----
    alloc_qk(0)
    for pc in (proj_pieces(0, 'q', 0) + proj_pieces(0, 'q', 1)
               + proj_pieces(0, 'k', 0) + proj_pieces(0, 'k', 1)
               + trans_pieces(0, 0) + trans_pieces(0, 1)):
        pc()

    # ---- pairs: attention with next pair's projections as in-loop filler ----
    pending = None
    for hp in range(KT):
        tg0 = trans_pieces(hp, 0) if hp > 0 else []
        tg1 = trans_pieces(hp, 1) if hp > 0 else []
        if hp + 1 < KT:
            alloc_qk(hp + 1)
            q0 = proj_pieces(hp + 1, 'q', 0)
            q1 = proj_pieces(hp + 1, 'q', 1)
            k0 = proj_pieces(hp + 1, 'k', 0)
            k1 = proj_pieces(hp + 1, 'k', 1)
        else:
            q0 = q1 = k0 = k1 = []
        pA = [pending] if pending else []
        fA = head_attn(hp, 2 * hp, tg0, pA + tg1 + q0 + q1)
        pB = [fA] if fA else []
        pending = head_attn(hp, 2 * hp + 1, [], pB + k0 + k1)
    if pending:
        pending()

    # ---- output projection: y = outT^T @ WoT + bo ----
    y_r = y_d.ap().rearrange("(st p) e -> st p e", p=P)
    for st in range(ST):
        ysb = ysb_pool.tile([P, E], F32, tag="ysb")
        for n0 in (0, 384):
            yps = ps_sc.tile([P, 512], F32, tag="ps_sc", name=f"yp_{st}_{n0}")
            for t in range(KT):
                nc.tensor.matmul(
                    yps[:, 0:384],
                    outT_sb[:, t, 128 * st:128 * st + 128],
                    WoT_sb[:, t, n0:n0 + 384],
                    start=(t == 0), stop=(t == KT - 1),
                )
            nc.vector.tensor_add(ysb[:, n0:n0 + 384], yps[:, 0:384],
                                 bo_bc[:, n0:n0 + 384])
        # stores ride the ACT hwdge queue so next iteration's input loads
        # on the sync queue are not serialized behind them
        nc.scalar.dma_start(y_r[st], ysb[:])


_NC_CACHE = {}


def build(iters=1, variant="full"):
    key = (iters, variant)
    nc = _NC_CACHE.get(key)
    if nc is None:
        nc = bacc.Bacc("TRN2", target_bir_lowering=False, debug=False)
        with tile.TileContext(nc) as tc, ExitStack() as ctx:
            _emit(nc, tc, ctx, iters=iters, variant=variant)
        nc.compile()
        _NC_CACHE[key] = nc
    return nc


def make_in_maps(x, Wq, bq, Wk, bk, Wo, bo):
    import ml_dtypes
    BF = ml_dtypes.bfloat16
    WqT = np.ascontiguousarray(np.asarray(Wq, dtype=np.float32).T).astype(BF)
    WkT = np.ascontiguousarray(np.asarray(Wk, dtype=np.float32).T).astype(BF)
    WoT = np.ascontiguousarray(np.asarray(Wo, dtype=np.float32).T).astype(BF)
    bq = np.ascontiguousarray(np.asarray(bq, dtype=np.float32))
    bk = np.ascontiguousarray(np.asarray(bk, dtype=np.float32))
    bo = np.ascontiguousarray(np.asarray(bo, dtype=np.float32))
    x = np.asarray(x, dtype=np.float32)
    return [
        {
            "xT": np.ascontiguousarray(x[c].T).astype(BF),
            "WqT": WqT, "WkT": WkT, "WoT": WoT,
            "bq": bq, "bk": bk, "bo": bo,
        }
        for c in range(NCORES)
    ]


def kernel(x, Wq, bq, Wk, bk, Wo, bo):
    nc = build()
    in_maps = make_in_maps(x, Wq, bq, Wk, bk, Wo, bo)
    res = bass_utils.run_bass_kernel_spmd(nc, in_maps, core_ids=list(range(NCORES)))
    return np.stack([res.results[c]["y"] for c in range(NCORES)]).astype(np.float32)


# revision 17
# speedup vs baseline: 1.4651x; 1.0213x over previous
"""Trainium2 Bass kernel for nn_Attention_86638080295542.

Multi-head attention (12 heads, d=64) with the reference's v=k quirk:
    q = x @ Wq.T + bq ; k = x @ Wk.T + bk ; v = k
    out = softmax(q k^T / sqrt(d)) @ v ;  y = out @ Wo.T + bo

Sharding: batch (B=8) data-parallel across the 8 NeuronCores — core c
computes batch element c end-to-end, no collectives.

Per-core dataflow (all "T" tensors keep the contraction dim on SBUF
partitions so every matmul is a natural lhsT.T @ rhs):
  xT[e,s], WqT/WkT/WoT[e_in,e_out] are pre-transposed on the host.
  qT = Wq @ xT   (+bq per-partition)        [768,1024]
  kT = Wk @ xT   (+bk per-partition)        [768,1024]
  vaug[j, jb, h, 0:64] = k natural (PE transpose of kT), col 64 = 1.0
  per head h: pT[j,i] = exp(scale * kT_h^T qT_h)  (no max-subtraction:
     logits are O(1) for this problem, softmax is shift-invariant)
  outT_h[d,i] (+ rowsum in row 64) = vaug^T @ pT, accumulated over j
  normalize: outT_h *= 1/rowsum (rowsum broadcast across the 64
     partitions via gpsimd partition_broadcast)
  y = outT^T @ WoT + bo

Schedule: the attention inner loop is ACT(exp)-paced, so the PE work
for the next pair's projections and this pair's vaug transposes is
split into ~2-matmul "pieces" and one piece is emitted per j-block
inside the attention loop, keeping the PE busy while ACT drains.  Exp
runs on full [128,1024] score tiles (fewer ACT instructions — HW has a
~150ns fixed cost per activation).  PV lags the scores by 3 j-blocks so
the single PV accumulator is free (previous head's normalization done)
before this head's first PV write.

PSUM budget (8 banks): scores ring 2x[128,1024] (4) + proj/trans ring
2x[128,512] (2) + one PV accumulator [65,1024] (2).
"""

from contextlib import ExitStack

import numpy as np

import concourse.bass as bass
import concourse.tile as tile
from concourse import bacc, mybir
from concourse import bass_utils

S = 1024          # sequence length
E = 768           # embed dim
H = 12            # heads
DH = 64           # head dim
P = 128           # partitions
KT = E // P       # 6 k-tiles over embed dim
ST = S // P       # 8 tiles over sequence
NCH = S // 512    # 2 free-dim chunks of 512 over sequence
SCALE = DH ** -0.5
NCORES = 8

F32 = mybir.dt.float32
BF16 = mybir.dt.bfloat16

# rowsum broadcast: 'gpsimd' = nc.gpsimd.partition_broadcast,
# 'dma' = DRAM round-trip with a 0-step partition AP
BCAST = 'gpsimd'


def _emit(nc, tc, ctx, iters=1, variant='full'):
    xT_d = nc.dram_tensor("xT", [E, S], BF16, kind="ExternalInput")
    WqT_d = nc.dram_tensor("WqT", [E, E], BF16, kind="ExternalInput")
    WkT_d = nc.dram_tensor("WkT", [E, E], BF16, kind="ExternalInput")
    WoT_d = nc.dram_tensor("WoT", [E, E], BF16, kind="ExternalInput")
    bq_d = nc.dram_tensor("bq", [E], F32, kind="ExternalInput")
    bk_d = nc.dram_tensor("bk", [E], F32, kind="ExternalInput")
    bo_d = nc.dram_tensor("bo", [E], F32, kind="ExternalInput")
    y_d = nc.dram_tensor("y", [S, E], F32, kind="ExternalOutput")

    Exp = mybir.ActivationFunctionType.Exp

    const = ctx.enter_context(tc.tile_pool(name="const", bufs=1))
    xt_pool = ctx.enter_context(tc.tile_pool(name="xt", bufs=1))
    outt_pool = ctx.enter_context(tc.tile_pool(name="outt", bufs=1))
    wq_pool = ctx.enter_context(tc.tile_pool(name="wq", bufs=1))
    wk_pool = ctx.enter_context(tc.tile_pool(name="wk", bufs=1))
    wo_pool = ctx.enter_context(tc.tile_pool(name="wo", bufs=1))
    vaug_pool = ctx.enter_context(tc.tile_pool(name="vaug", bufs=1))
    qt_pool = ctx.enter_context(tc.tile_pool(name="qt", bufs=3))
    kt_pool = ctx.enter_context(tc.tile_pool(name="kt", bufs=3))
    pt_pool = ctx.enter_context(tc.tile_pool(name="pt", bufs=8))
    ysb_pool = ctx.enter_context(tc.tile_pool(name="ysb", bufs=4))
    rc_pool = ctx.enter_context(tc.tile_pool(name="rc", bufs=2))
    rb_pool = ctx.enter_context(tc.tile_pool(name="rb", bufs=2))
    ps_sc = ctx.enter_context(tc.tile_pool(name="ps_sc", bufs=2, space="PSUM"))
    ps_w = ctx.enter_context(tc.tile_pool(name="ps_w", bufs=2, space="PSUM"))
    ps_pv = ctx.enter_context(tc.tile_pool(name="ps_pv", bufs=1, space="PSUM"))
    if BCAST == 'dma':
        dram_pool = ctx.enter_context(
            tc.tile_pool(name="dram", bufs=4, space="DRAM"))

    # ---- loop-invariant constants (outside the timing loop) ----
    ident_f32 = const.tile([P, P], F32, tag="ident_f32")
    from concourse.masks import make_identity
    make_identity(nc, ident_f32[:])
    identity = const.tile([P, P], BF16, tag="ident")
    nc.vector.tensor_copy(identity[:], ident_f32[:])
    bq_sb = const.tile([P, KT], F32, tag="bq")
    nc.sync.dma_start(bq_sb[:], bq_d.ap().rearrange("(t p) -> p t", p=P))
    bk_sb = const.tile([P, KT], F32, tag="bk")
    nc.sync.dma_start(bk_sb[:], bk_d.ap().rearrange("(t p) -> p t", p=P))
    # bo broadcast to all 128 partitions via a 0-step partition AP (DRAM APs
    # are not partitioned, so a 0-step leading dim is legal here)
    bo_bc = const.tile([P, E], F32, tag="bo")
    bo_ap = bo_d.ap()
    bo_bcast_src = bass.AP(bo_ap.tensor, bo_ap.offset, [[0, P], [1, E]])
    nc.sync.dma_start(bo_bc[:], bo_bcast_src)

    if iters > 1:
        ctx.enter_context(tc.For_i(0, iters, 1))

    # ---- input loads: xT first (needed in full by proj 0), Wq/Wk in
    # hp-column slices so proj hp only waits for slice hp, WoT last ----
    xT_sb = xt_pool.tile([P, KT, S], BF16, tag="xt")
    WqT_sb = wq_pool.tile([P, KT, E], BF16, tag="wq")
    WkT_sb = wk_pool.tile([P, KT, E], BF16, tag="wk")
    WoT_sb = wo_pool.tile([P, KT, E], BF16, tag="wo")
    xT_r = xT_d.ap().rearrange("(t p) s -> p t s", p=P)
    WqT_r = WqT_d.ap().rearrange("(t p) e -> p t e", p=P)
    WkT_r = WkT_d.ap().rearrange("(t p) e -> p t e", p=P)
    WoT_r = WoT_d.ap().rearrange("(t p) e -> p t e", p=P)
    nc.sync.dma_start(xT_sb[:, 0, :], xT_r[:, 0, :])
    nc.sync.dma_start(WqT_sb[:, :, 0:P], WqT_r[:, :, 0:P])
    nc.sync.dma_start(WkT_sb[:, :, 0:P], WkT_r[:, :, 0:P])
    for t in range(1, KT):
        nc.sync.dma_start(xT_sb[:, t, :], xT_r[:, t, :])
    for hp in range(1, KT):
        c0, c1 = P * hp, P * hp + P
        nc.sync.dma_start(WqT_sb[:, :, c0:c1], WqT_r[:, :, c0:c1])
        nc.sync.dma_start(WkT_sb[:, :, c0:c1], WkT_r[:, :, c0:c1])
    for t in range(KT):
        nc.sync.dma_start(WoT_sb[:, t, :], WoT_r[:, t, :])

    vaug = vaug_pool.tile([P, ST, H, DH + 1], BF16, tag="vaug")
    for jb in range(ST):
        nc.vector.memset(vaug[:, jb, :, DH:DH + 1], 1.0)
    outT_sb = outt_pool.tile([P, KT, S], BF16, tag="outt")

    if variant == 'noexp':
        pt_const = const.tile([P, S], BF16, tag="ptc")
        nc.vector.memset(pt_const[:], 0.01)
        scr = const.tile([P, 256], F32, tag="scr")

    qps = [None] * KT
    kps = [None] * KT

    def proj_pieces(hp, which, c):
        """One projection chunk as 3 pieces of 2 accumulating mms each; the
        last piece evicts to SBUF with the bias add (DVE)."""
        W_sb, b_sb = (WqT_sb, bq_sb) if which == 'q' else (WkT_sb, bk_sb)
        out_sb = qps[hp] if which == 'q' else kps[hp]
        st = {}

        def piece(tp, first, last):
            def go():
                if first:
                    st['ps'] = ps_w.tile([P, 512], F32, tag="ps_w",
                                         name=f"pj_{which}{hp}_{c}")
                ps = st['ps']
                for t in tp:
                    nc.tensor.matmul(
                        ps[:],
                        W_sb[:, t, 128 * hp:128 * hp + 128],
                        xT_sb[:, t, 512 * c:512 * c + 512],
                        start=(t == 0), stop=(t == KT - 1),
                    )
                if last:
                    nc.vector.tensor_scalar_add(
                        out_sb[:, 512 * c:512 * c + 512], ps[:],
                        b_sb[:, hp:hp + 1])
            return go

        return [piece((0, 1), True, False), piece((2, 3), False, False),
                piece((4, 5), False, True)]

    def trans_pieces(hp, g):
        """4 PE transposes of kp(hp) block g + DVE copy into vaug, as 2
        pieces of 2 transposes each."""
        st = {}

        def piece(j4s, first, last):
            def go():
                if first:
                    st['ps'] = ps_w.tile([P, 512], BF16, tag="ps_w",
                                         name=f"tr_{hp}_{g}")
                ps = st['ps']
                kp = kps[hp]
                for j4 in j4s:
                    jb = 4 * g + j4
                    nc.tensor.transpose(
                        ps[:, 128 * j4:128 * j4 + 128],
                        kp[:, 128 * jb:128 * jb + 128],
                        identity[:],
                    )
                if last:
                    nc.vector.tensor_copy(
                        vaug[:, 4 * g:4 * g + 4, 2 * hp:2 * hp + 2, 0:DH],
                        ps[:].rearrange("p (a b c) -> p a b c", a=4, b=2, c=DH),
                    )
            return go

        return [piece((0, 1), True, False), piece((2, 3), False, True)]

    def alloc_qk(hp):
        qps[hp] = qt_pool.tile([P, S], BF16, tag="qt", name=f"qp_{hp}")
        kps[hp] = kt_pool.tile([P, S], BF16, tag="kt", name=f"kp_{hp}")

    def head_attn(hp, h, pre, fill):
        """Attention for head h of pair hp. `pre` pieces run before the
        j-loop; `fill` pieces are consumed one per j-block inside it."""
        po = DH * (h % 2)
        if variant == 'noattn':
            for f in pre:
                f()
            for f in fill:
                f()
            if h % 2 == 0:
                nc.vector.memset(outT_sb[:, hp, :], 0.01)
            return None
        qp, kp = qps[hp], kps[hp]
        pv = ps_pv.tile([DH + 1, S], F32, tag="ps_pv", name=f"pv_{h}")
        for f in pre:
            f()

        def pv_mms(jb, pt):
            for c in range(NCH):
                nc.tensor.matmul(
                    pv[:, 512 * c:512 * c + 512],
                    vaug[:, jb, h, :],
                    pt[:, 512 * c:512 * c + 512],
                    start=(jb == 0), stop=(jb == ST - 1),
                )

        LAG = 3
        fi = 0
        pts = []
        for jb in range(ST):
            pt = pt_pool.tile([P, S], BF16, tag="pt")
            sps = ps_sc.tile([P, S], F32, tag="ps_sc", name=f"sps_{h}_{jb}")
            for c in range(NCH):
                nc.tensor.matmul(
                    sps[:, 512 * c:512 * c + 512],
                    kp[po:po + DH, 128 * jb:128 * jb + 128],
                    qp[po:po + DH, 512 * c:512 * c + 512],
                    start=True, stop=True,
                )
            if variant == 'noexp':
                # skeleton timing: keep scores mms (cheap reader frees the
                # ring slot), drop the ACT dependency from the PV chain
                nc.vector.tensor_copy(scr[:, 2 * (jb % 8):2 * (jb % 8) + 1],
                                      sps[:, 0:1])
                pt = pt_const
            else:
                nc.scalar.activation(pt[:], sps[:], Exp, scale=SCALE)
            pts.append(pt)
            if fi < len(fill):
                fill[fi]()
                fi += 1
            if jb >= LAG:
                pv_mms(jb - LAG, pts[jb - LAG])
        for jb in range(ST - LAG, ST):
            pv_mms(jb, pts[jb])
        for f in fill[fi:]:
            f()

        # normalization, chunked so the chain latency is ~half a tile:
        # rc = 1/rowsum (DVE, straight off PSUM), broadcast across the 64
        # head-dim partitions on gpsimd.  The multiplies are returned as a
        # closure and emitted early in the NEXT head's fill schedule, so
        # their wait on the broadcast does not head-of-line-block the DVE
        # FIFO behind them (psum-freeing evictions).
        rc = rc_pool.tile([1, S], F32, tag="rc", name=f"rc_{h}")
        rb = rb_pool.tile([DH, S], F32, tag="rb", name=f"rb_{h}")
        if BCAST == 'dma':
            rd = dram_pool.tile([1, S], F32, tag="rd", name=f"rd_{h}")
        for c in range(NCH):
            cs = slice(512 * c, 512 * c + 512)
            nc.vector.reciprocal(rc[:, cs], pv[DH:DH + 1, cs])
            if BCAST == 'gpsimd':
                nc.gpsimd.partition_broadcast(rb[:, cs], rc[:, cs])
            else:
                nc.sync.dma_start(rd[:, cs], rc[:, cs])
                rd_ap = rd[:, cs]
                nc.sync.dma_start(
                    rb[:, cs],
                    bass.AP(rd_ap.tensor, rd_ap.offset, [[0, DH], [1, 512]]))

        def finish_norm():
            for c in range(NCH):
                cs = slice(512 * c, 512 * c + 512)
                nc.vector.tensor_mul(
                    outT_sb[po:po + DH, hp, cs], pv[0:DH, cs], rb[:, cs])
        return finish_norm

    # ---- pair 0 projections (no attention to hide them under) ----
    alloc_qk(0)
    for pc in (proj_pieces(0, 'q', 0) + proj_pieces(0, 'q', 1)
               + proj_pieces(0, 'k', 0) + proj_pieces(0, 'k', 1)
               + trans_pieces(0, 0) + trans_pieces(0, 1)):
        pc()

    # ---- pairs: attention with next pair's projections as in-loop filler ----
    pending = None
    for hp in range(KT):
        tg0 = trans_pieces(hp, 0) if hp > 0 else []
        tg1 = trans_pieces(hp, 1) if hp > 0 else []
        if hp + 1 < KT:
            alloc_qk(hp + 1)
            q0 = proj_pieces(hp + 1, 'q', 0)
            q1 = proj_pieces(hp + 1, 'q', 1)
            k0 = proj_pieces(hp + 1, 'k', 0)
            k1 = proj_pieces(hp + 1, 'k', 1)
        else:
            q0 = q1 = k0 = k1 = []
        pA = [pending] if pending else []
        fA = head_attn(hp, 2 * hp, tg0, pA + tg1 + q0 + q1[:2])
        pB = [fA] if fA else []
        pending = head_attn(hp, 2 * hp + 1, [], pB + q1[2:] + k0 + k1)
    if pending:
        pending()

    # ---- output projection: y = outT^T @ WoT + bo ----
    y_r = y_d.ap().rearrange("(st p) e -> st p e", p=P)
    for st in range(ST):
        ysb = ysb_pool.tile([P, E], F32, tag="ysb")
        for n0 in (0, 384):
            yps = ps_sc.tile([P, 512], F32, tag="ps_sc", name=f"yp_{st}_{n0}")
            for t in range(KT):
                nc.tensor.matmul(
                    yps[:, 0:384],
                    outT_sb[:, t, 128 * st:128 * st + 128],
                    WoT_sb[:, t, n0:n0 + 384],
                    start=(t == 0), stop=(t == KT - 1),
                )
            nc.vector.tensor_add(ysb[:, n0:n0 + 384], yps[:, 0:384],
                                 bo_bc[:, n0:n0 + 384])
        # stores ride the ACT hwdge queue so next iteration's input loads
        # on the sync queue are not serialized behind them
        nc.scalar.dma_start(y_r[st], ysb[:])


_NC_CACHE = {}


def build(iters=1, variant="full"):
    key = (iters, variant)
    nc = _NC_CACHE.get(key)
    if nc is None:
        nc = bacc.Bacc("TRN2", target_bir_lowering=False, debug=False)
        with tile.TileContext(nc) as tc, ExitStack() as ctx:
            _emit(nc, tc, ctx, iters=iters, variant=variant)
        nc.compile()
        _NC_CACHE[key] = nc
    return nc


def make_in_maps(x, Wq, bq, Wk, bk, Wo, bo):
    import ml_dtypes
    BF = ml_dtypes.bfloat16
    WqT = np.ascontiguousarray(np.asarray(Wq, dtype=np.float32).T).astype(BF)
    WkT = np.ascontiguousarray(np.asarray(Wk, dtype=np.float32).T).astype(BF)
    WoT = np.ascontiguousarray(np.asarray(Wo, dtype=np.float32).T).astype(BF)
    bq = np.ascontiguousarray(np.asarray(bq, dtype=np.float32))
    bk = np.ascontiguousarray(np.asarray(bk, dtype=np.float32))
    bo = np.ascontiguousarray(np.asarray(bo, dtype=np.float32))
    x = np.asarray(x, dtype=np.float32)
    return [
        {
            "xT": np.ascontiguousarray(x[c].T).astype(BF),
            "WqT": WqT, "WkT": WkT, "WoT": WoT,
            "bq": bq, "bk": bk, "bo": bo,
        }
        for c in range(NCORES)
    ]


def kernel(x, Wq, bq, Wk, bk, Wo, bo):
    nc = build()
    in_maps = make_in_maps(x, Wq, bq, Wk, bk, Wo, bo)
    res = bass_utils.run_bass_kernel_spmd(nc, in_maps, core_ids=list(range(NCORES)))
    return np.stack([res.results[c]["y"] for c in range(NCORES)]).astype(np.float32)


# revision 22
# speedup vs baseline: 1.4698x; 1.0032x over previous
"""Trainium2 Bass kernel for nn_Attention_86638080295542.

Multi-head attention (12 heads, d=64) with the reference's v=k quirk:
    q = x @ Wq.T + bq ; k = x @ Wk.T + bk ; v = k
    out = softmax(q k^T / sqrt(d)) @ v ;  y = out @ Wo.T + bo

Sharding: batch (B=8) data-parallel across the 8 NeuronCores — core c
computes batch element c end-to-end, no collectives.

Per-core dataflow (all "T" tensors keep the contraction dim on SBUF
partitions so every matmul is a natural lhsT.T @ rhs):
  xT[e,s], WqT/WkT/WoT[e_in,e_out] are pre-transposed on the host.
  qT = Wq @ xT   (+bq per-partition)        [768,1024]
  kT = Wk @ xT   (+bk per-partition)        [768,1024]
  vaug[j, jb, h, 0:64] = k natural (PE transpose of kT), col 64 = 1.0
  per head h: pT[j,i] = exp(scale * kT_h^T qT_h)  (no max-subtraction:
     logits are O(1) for this problem, softmax is shift-invariant)
  outT_h[d,i] (+ rowsum in row 64) = vaug^T @ pT, accumulated over j
  normalize: outT_h *= 1/rowsum (rowsum broadcast across the 64
     partitions via gpsimd partition_broadcast)
  y = outT^T @ WoT + bo

Schedule: the attention inner loop is ACT(exp)-paced, so the PE work
for the next pair's projections and this pair's vaug transposes is
split into ~2-matmul "pieces" and one piece is emitted per j-block
inside the attention loop, keeping the PE busy while ACT drains.  Exp
runs on full [128,1024] score tiles (fewer ACT instructions — HW has a
~150ns fixed cost per activation).  PV lags the scores by 3 j-blocks so
the single PV accumulator is free (previous head's normalization done)
before this head's first PV write.

PSUM budget (8 banks): scores ring 2x[128,1024] (4) + proj/trans ring
2x[128,512] (2) + one PV accumulator [65,1024] (2).
"""

from contextlib import ExitStack

import numpy as np

import concourse.bass as bass
import concourse.tile as tile
from concourse import bacc, mybir
from concourse import bass_utils

S = 1024          # sequence length
E = 768           # embed dim
H = 12            # heads
DH = 64           # head dim
P = 128           # partitions
KT = E // P       # 6 k-tiles over embed dim
ST = S // P       # 8 tiles over sequence
NCH = S // 512    # 2 free-dim chunks of 512 over sequence
SCALE = DH ** -0.5
NCORES = 8

F32 = mybir.dt.float32
BF16 = mybir.dt.bfloat16

# rowsum broadcast: 'gpsimd' = nc.gpsimd.partition_broadcast,
# 'dma' = DRAM round-trip with a 0-step partition AP
BCAST = 'gpsimd'


def _emit(nc, tc, ctx, iters=1, variant='full'):
    xT_d = nc.dram_tensor("xT", [E, S], BF16, kind="ExternalInput")
    WqT_d = nc.dram_tensor("WqT", [E, E], BF16, kind="ExternalInput")
    WkT_d = nc.dram_tensor("WkT", [E, E], BF16, kind="ExternalInput")
    WoT_d = nc.dram_tensor("WoT", [E, E], BF16, kind="ExternalInput")
    bq_d = nc.dram_tensor("bq", [E], F32, kind="ExternalInput")
    bk_d = nc.dram_tensor("bk", [E], F32, kind="ExternalInput")
    bo_d = nc.dram_tensor("bo", [E], F32, kind="ExternalInput")
    y_d = nc.dram_tensor("y", [S, E], F32, kind="ExternalOutput")

    Exp = mybir.ActivationFunctionType.Exp

    const = ctx.enter_context(tc.tile_pool(name="const", bufs=1))
    xt_pool = ctx.enter_context(tc.tile_pool(name="xt", bufs=1))
    outt_pool = ctx.enter_context(tc.tile_pool(name="outt", bufs=1))
    wq_pool = ctx.enter_context(tc.tile_pool(name="wq", bufs=1))
    wk_pool = ctx.enter_context(tc.tile_pool(name="wk", bufs=1))
    wo_pool = ctx.enter_context(tc.tile_pool(name="wo", bufs=1))
    vaug_pool = ctx.enter_context(tc.tile_pool(name="vaug", bufs=1))
    qt_pool = ctx.enter_context(tc.tile_pool(name="qt", bufs=3))
    kt_pool = ctx.enter_context(tc.tile_pool(name="kt", bufs=3))
    pt_pool = ctx.enter_context(tc.tile_pool(name="pt", bufs=8))
    ysb_pool = ctx.enter_context(tc.tile_pool(name="ysb", bufs=4))
    rc_pool = ctx.enter_context(tc.tile_pool(name="rc", bufs=2))
    rb_pool = ctx.enter_context(tc.tile_pool(name="rb", bufs=2))
    ps_sc = ctx.enter_context(tc.tile_pool(name="ps_sc", bufs=2, space="PSUM"))
    ps_w = ctx.enter_context(tc.tile_pool(name="ps_w", bufs=2, space="PSUM"))
    ps_pv = ctx.enter_context(tc.tile_pool(name="ps_pv", bufs=1, space="PSUM"))
    if BCAST == 'dma':
        dram_pool = ctx.enter_context(
            tc.tile_pool(name="dram", bufs=4, space="DRAM"))

    # ---- loop-invariant constants (outside the timing loop) ----
    ident_f32 = const.tile([P, P], F32, tag="ident_f32")
    from concourse.masks import make_identity
    make_identity(nc, ident_f32[:])
    identity = const.tile([P, P], BF16, tag="ident")
    nc.vector.tensor_copy(identity[:], ident_f32[:])
    bq_sb = const.tile([P, KT], F32, tag="bq")
    nc.sync.dma_start(bq_sb[:], bq_d.ap().rearrange("(t p) -> p t", p=P))
    bk_sb = const.tile([P, KT], F32, tag="bk")
    nc.sync.dma_start(bk_sb[:], bk_d.ap().rearrange("(t p) -> p t", p=P))
    # bo broadcast to all 128 partitions via a 0-step partition AP (DRAM APs
    # are not partitioned, so a 0-step leading dim is legal here)
    bo_bc = const.tile([P, E], F32, tag="bo")
    bo_ap = bo_d.ap()
    bo_bcast_src = bass.AP(bo_ap.tensor, bo_ap.offset, [[0, P], [1, E]])
    nc.sync.dma_start(bo_bc[:], bo_bcast_src)

    if iters > 1:
        ctx.enter_context(tc.For_i(0, iters, 1))

    # ---- input loads: xT first (needed in full by proj 0), Wq/Wk in
    # hp-column slices so proj hp only waits for slice hp, WoT last ----
    xT_sb = xt_pool.tile([P, KT, S], BF16, tag="xt")
    WqT_sb = wq_pool.tile([P, KT, E], BF16, tag="wq")
    WkT_sb = wk_pool.tile([P, KT, E], BF16, tag="wk")
    WoT_sb = wo_pool.tile([P, KT, E], BF16, tag="wo")
    xT_r = xT_d.ap().rearrange("(t p) s -> p t s", p=P)
    WqT_r = WqT_d.ap().rearrange("(t p) e -> p t e", p=P)
    WkT_r = WkT_d.ap().rearrange("(t p) e -> p t e", p=P)
    WoT_r = WoT_d.ap().rearrange("(t p) e -> p t e", p=P)
    nc.sync.dma_start(xT_sb[:, 0, :], xT_r[:, 0, :])
    nc.sync.dma_start(WqT_sb[:, :, 0:P], WqT_r[:, :, 0:P])
    nc.sync.dma_start(WkT_sb[:, :, 0:P], WkT_r[:, :, 0:P])
    for t in range(1, KT):
        nc.sync.dma_start(xT_sb[:, t, :], xT_r[:, t, :])
    for hp in range(1, KT):
        c0, c1 = P * hp, P * hp + P
        nc.sync.dma_start(WqT_sb[:, :, c0:c1], WqT_r[:, :, c0:c1])
        nc.sync.dma_start(WkT_sb[:, :, c0:c1], WkT_r[:, :, c0:c1])
    for t in range(KT):
        nc.sync.dma_start(WoT_sb[:, t, :], WoT_r[:, t, :])

    vaug = vaug_pool.tile([P, ST, H, DH + 1], BF16, tag="vaug")
    for jb in range(ST):
        nc.vector.memset(vaug[:, jb, :, DH:DH + 1], 1.0)
    outT_sb = outt_pool.tile([P, KT, S], BF16, tag="outt")

    if variant == 'noexp':
        pt_const = const.tile([P, S], BF16, tag="ptc")
        nc.vector.memset(pt_const[:], 0.01)
        scr = const.tile([P, 256], F32, tag="scr")

    qps = [None] * KT
    kps = [None] * KT

    def proj_pieces(hp, which, c):
        """One projection chunk as 3 pieces of 2 accumulating mms each; the
        last piece evicts to SBUF with the bias add (DVE)."""
        W_sb, b_sb = (WqT_sb, bq_sb) if which == 'q' else (WkT_sb, bk_sb)
        out_sb = qps[hp] if which == 'q' else kps[hp]
        st = {}

        def piece(tp, first, last):
            def go():
                if first:
                    st['ps'] = ps_w.tile([P, 512], F32, tag="ps_w",
                                         name=f"pj_{which}{hp}_{c}")
                ps = st['ps']
                for t in tp:
                    nc.tensor.matmul(
                        ps[:],
                        W_sb[:, t, 128 * hp:128 * hp + 128],
                        xT_sb[:, t, 512 * c:512 * c + 512],
                        start=(t == 0), stop=(t == KT - 1),
                    )
                if last:
                    nc.vector.tensor_scalar_add(
                        out_sb[:, 512 * c:512 * c + 512], ps[:],
                        b_sb[:, hp:hp + 1])
            return go

        return [piece((0, 1), True, False), piece((2, 3), False, False),
                piece((4, 5), False, True)]

    def trans_pieces(hp, g):
        """4 PE transposes of kp(hp) block g + DVE copy into vaug, as 2
        pieces of 2 transposes each."""
        st = {}

        def piece(j4s, first, last):
            def go():
                if first:
                    st['ps'] = ps_w.tile([P, 512], BF16, tag="ps_w",
                                         name=f"tr_{hp}_{g}")
                ps = st['ps']
                kp = kps[hp]
                for j4 in j4s:
                    jb = 4 * g + j4
                    nc.tensor.transpose(
                        ps[:, 128 * j4:128 * j4 + 128],
                        kp[:, 128 * jb:128 * jb + 128],
                        identity[:],
                    )
                if last:
                    nc.vector.tensor_copy(
                        vaug[:, 4 * g:4 * g + 4, 2 * hp:2 * hp + 2, 0:DH],
                        ps[:].rearrange("p (a b c) -> p a b c", a=4, b=2, c=DH),
                    )
            return go

        return [piece((0, 1), True, False), piece((2, 3), False, True)]

    def alloc_qk(hp):
        qps[hp] = qt_pool.tile([P, S], BF16, tag="qt", name=f"qp_{hp}")
        kps[hp] = kt_pool.tile([P, S], BF16, tag="kt", name=f"kp_{hp}")

    def yp_partial_pieces(st, n0, holder):
        """Pre-accumulate outproj chunk (st, n0) over t=0..4 in a ps_w slot,
        as pair-5 filler (its slots are otherwise idle there).  The t=5
        matmul + stop runs in the outproj section after the last norm."""
        def piece(ts, first):
            def go():
                if first:
                    holder['ps'] = ps_w.tile([P, 512], F32, tag="ps_w",
                                             name=f"yppre_{st}_{n0}")
                for t in ts:
                    nc.tensor.matmul(
                        holder['ps'][:, 0:384],
                        outT_sb[:, t, 128 * st:128 * st + 128],
                        WoT_sb[:, t, n0:n0 + 384],
                        start=(t == 0), stop=False,
                    )
            return go
        return [piece((0, 1), True), piece((2, 3), False), piece((4,), False)]

    def head_attn(hp, h, pre, fill):
        """Attention for head h of pair hp. `pre` pieces run before the
        j-loop; `fill` pieces are consumed one per j-block inside it."""
        po = DH * (h % 2)
        if variant == 'noattn':
            for f in pre:
                f()
            for f in fill:
                f()
            if h % 2 == 0:
                nc.vector.memset(outT_sb[:, hp, :], 0.01)
            return None
        qp, kp = qps[hp], kps[hp]
        pv = ps_pv.tile([DH + 1, S], F32, tag="ps_pv", name=f"pv_{h}")
        for f in pre:
            f()

        def pv_mms(jb, pt):
            for c in range(NCH):
                nc.tensor.matmul(
                    pv[:, 512 * c:512 * c + 512],
                    vaug[:, jb, h, :],
                    pt[:, 512 * c:512 * c + 512],
                    start=(jb == 0), stop=(jb == ST - 1),
                )

        LAG = 3
        fi = 0
        pts = []
        for jb in range(ST):
            pt = pt_pool.tile([P, S], BF16, tag="pt")
            sps = ps_sc.tile([P, S], F32, tag="ps_sc", name=f"sps_{h}_{jb}")
            for c in range(NCH):
                nc.tensor.matmul(
                    sps[:, 512 * c:512 * c + 512],
                    kp[po:po + DH, 128 * jb:128 * jb + 128],
                    qp[po:po + DH, 512 * c:512 * c + 512],
                    start=True, stop=True,
                )
            if variant == 'noexp':
                # skeleton timing: keep scores mms (cheap reader frees the
                # ring slot), drop the ACT dependency from the PV chain
                nc.vector.tensor_copy(scr[:, 2 * (jb % 8):2 * (jb % 8) + 1],
                                      sps[:, 0:1])
                pt = pt_const
            else:
                nc.scalar.activation(pt[:], sps[:], Exp, scale=SCALE)
            pts.append(pt)
            if fi < len(fill):
                fill[fi]()
                fi += 1
            if jb >= LAG:
                pv_mms(jb - LAG, pts[jb - LAG])
        for jb in range(ST - LAG, ST):
            pv_mms(jb, pts[jb])
        for f in fill[fi:]:
            f()

        # normalization, chunked so the chain latency is ~half a tile:
        # rc = 1/rowsum (DVE, straight off PSUM), broadcast across the 64
        # head-dim partitions on gpsimd.  The multiplies are returned as a
        # closure and emitted early in the NEXT head's fill schedule, so
        # their wait on the broadcast does not head-of-line-block the DVE
        # FIFO behind them (psum-freeing evictions).
        rc = rc_pool.tile([1, S], F32, tag="rc", name=f"rc_{h}")
        rb = rb_pool.tile([DH, S], F32, tag="rb", name=f"rb_{h}")
        if BCAST == 'dma':
            rd = dram_pool.tile([1, S], F32, tag="rd", name=f"rd_{h}")
        for c in range(NCH):
            cs = slice(512 * c, 512 * c + 512)
            nc.vector.reciprocal(rc[:, cs], pv[DH:DH + 1, cs])
            if BCAST == 'gpsimd':
                nc.gpsimd.partition_broadcast(rb[:, cs], rc[:, cs])
            else:
                nc.sync.dma_start(rd[:, cs], rc[:, cs])
                rd_ap = rd[:, cs]
                nc.sync.dma_start(
                    rb[:, cs],
                    bass.AP(rd_ap.tensor, rd_ap.offset, [[0, DH], [1, 512]]))

        def finish_norm():
            for c in range(NCH):
                cs = slice(512 * c, 512 * c + 512)
                nc.vector.tensor_mul(
                    outT_sb[po:po + DH, hp, cs], pv[0:DH, cs], rb[:, cs])
        return finish_norm

    # ---- pair 0 projections (no attention to hide them under) ----
    alloc_qk(0)
    for pc in (proj_pieces(0, 'q', 0) + proj_pieces(0, 'q', 1)
               + proj_pieces(0, 'k', 0) + proj_pieces(0, 'k', 1)
               + trans_pieces(0, 0) + trans_pieces(0, 1)):
        pc()

    # ---- pairs: attention with next pair's projections as in-loop filler ----
    pending = None
    yp_pre = [{}, {}]
    for hp in range(KT):
        tg0 = trans_pieces(hp, 0) if hp > 0 else []
        tg1 = trans_pieces(hp, 1) if hp > 0 else []
        if hp + 1 < KT:
            alloc_qk(hp + 1)
            q0 = proj_pieces(hp + 1, 'q', 0)
            q1 = proj_pieces(hp + 1, 'q', 1)
            k0 = proj_pieces(hp + 1, 'k', 0)
            k1 = proj_pieces(hp + 1, 'k', 1)
        else:
            q0 = q1 = k0 = k1 = []
        if hp + 1 < KT or variant == 'noattn':
            ypA, ypB = [], []
        else:
            ypA = yp_partial_pieces(0, 0, yp_pre[0])
            ypB = yp_partial_pieces(0, 384, yp_pre[1])
        pA = [pending] if pending else []
        fA = head_attn(hp, 2 * hp, tg0, pA + tg1 + q0 + q1[:2] + ypA)
        pB = [fA] if fA else []
        pending = head_attn(hp, 2 * hp + 1, [],
                            pB + q1[2:] + k0 + k1 + ypB)
    if pending:
        pending()

    # ---- output projection: y = outT^T @ WoT + bo ----
    y_r = y_d.ap().rearrange("(st p) e -> st p e", p=P)
    for st in range(ST):
        ysb = ysb_pool.tile([P, E], F32, tag="ysb")
        for n0 in (0, 384):
            pre = yp_pre[0 if n0 == 0 else 1] if (
                st == 0 and variant != 'noattn') else None
            if pre and 'ps' in pre:
                yps = pre['ps']
                nc.tensor.matmul(
                    yps[:, 0:384],
                    outT_sb[:, KT - 1, 128 * st:128 * st + 128],
                    WoT_sb[:, KT - 1, n0:n0 + 384],
                    start=False, stop=True,
                )
            else:
                yps = ps_sc.tile([P, 512], F32, tag="ps_sc",
                                 name=f"yp_{st}_{n0}")
                for t in range(KT):
                    nc.tensor.matmul(
                        yps[:, 0:384],
                        outT_sb[:, t, 128 * st:128 * st + 128],
                        WoT_sb[:, t, n0:n0 + 384],
                        start=(t == 0), stop=(t == KT - 1),
                    )
            nc.vector.tensor_add(ysb[:, n0:n0 + 384], yps[:, 0:384],
                                 bo_bc[:, n0:n0 + 384])
        # stores ride the ACT hwdge queue so next iteration's input loads
        # on the sync queue are not serialized behind them
        nc.scalar.dma_start(y_r[st], ysb[:])


_NC_CACHE = {}


def build(iters=1, variant="full"):
    key = (iters, variant)
    nc = _NC_CACHE.get(key)
    if nc is None:
        nc = bacc.Bacc("TRN2", target_bir_lowering=False, debug=False)
        with tile.TileContext(nc) as tc, ExitStack() as ctx:
            _emit(nc, tc, ctx, iters=iters, variant=variant)
        nc.compile()
        _NC_CACHE[key] = nc
    return nc


def make_in_maps(x, Wq, bq, Wk, bk, Wo, bo):
    import ml_dtypes
    BF = ml_dtypes.bfloat16
    WqT = np.ascontiguousarray(np.asarray(Wq, dtype=np.float32).T).astype(BF)
    WkT = np.ascontiguousarray(np.asarray(Wk, dtype=np.float32).T).astype(BF)
    WoT = np.ascontiguousarray(np.asarray(Wo, dtype=np.float32).T).astype(BF)
    bq = np.ascontiguousarray(np.asarray(bq, dtype=np.float32))
    bk = np.ascontiguousarray(np.asarray(bk, dtype=np.float32))
    bo = np.ascontiguousarray(np.asarray(bo, dtype=np.float32))
    x = np.asarray(x, dtype=np.float32)
    return [
        {
            "xT": np.ascontiguousarray(x[c].T).astype(BF),
            "WqT": WqT, "WkT": WkT, "WoT": WoT,
            "bq": bq, "bk": bk, "bo": bo,
        }
        for c in range(NCORES)
    ]


def kernel(x, Wq, bq, Wk, bk, Wo, bo):
    nc = build()
    in_maps = make_in_maps(x, Wq, bq, Wk, bk, Wo, bo)
    res = bass_utils.run_bass_kernel_spmd(nc, in_maps, core_ids=list(range(NCORES)))
    return np.stack([res.results[c]["y"] for c in range(NCORES)]).astype(np.float32)
